# revision 62
# baseline (speedup 1.0000x reference)
"""MaskClusterAttention Trainium2 kernel (fp8 DoubleRow redesign).

Sparse attention: tokens attend only within their cluster (64 clusters,
~64 tokens each).  Host sorts clusters by size and assigns rank
8j+c -> (core c, block j), so all 8 cores share one SPMD program with
per-block compile-time widths S_j = max size in rank-group j (>=64,
mult of 4).  Per-core padding is ~9% instead of the 50% a fixed 96-pad
costs.

Per core (T = sum S_j tokens):
  S1: q,k projections emitted transposed [128, T] via fp8e4 DoubleRow
      matmuls (2x128 contraction per instruction, 0.5 cyc/row); v per
      block in [S_j, 1024] layout.  x is staged as interleaved fp8
      hi/lo pairs [128, 8, 2, T] so single-fp8 (hi) and compensated
      (hi+lo) passes share one layout.  q scaled by 1/sqrt(hd) and in_proj
      bias applied during the PSUM->SBUF copy (Act, per-partition bias).
  S2: per (head-pair, block): scoresT [S_j, 2, S_j] in PSUM; pad-key
      masking is a rank-1 matmul (padflag x -1e4 row) accumulated into
      the same PSUM group, so exp needs no bias and merges 4 blocks per
      Act op.  Row sums via gpsimd partition_all_reduce, reciprocal
      (DVE, bf16), normalize-multiply fused with the fp8 downcast of
      ctxT.
  S3: out_proj via fp8 DoubleRow from ctxT8; residual x and bias bo'
      (bo + bv@Wo, v-bias folded out) enter the same PSUM via identity /
      ones rank-1 matmuls.  LayerNorm stats come free from Act accum_out
      (sum y, sum y^2), apply via one tensor_scalar.  No collectives.

Host scatters per-core outputs back through the slot map.
"""

from contextlib import ExitStack

import ml_dtypes
import numpy as np

import concourse.bass as bass
import concourse.bass_isa as bass_isa
import concourse.mybir as mybir
import concourse.tile as tile
from concourse import bacc
from concourse.bass_utils import run_bass_kernel_spmd

F32 = mybir.dt.float32
BF16 = mybir.dt.bfloat16
FP8 = mybir.dt.float8e4
NFP8 = ml_dtypes.float8_e4m3
NBF16 = ml_dtypes.bfloat16

N, D, H, HD, NCLUST, NCORES = 4096, 1024, 16, 64, 64, 8
NB = NCLUST // NCORES  # blocks (clusters) per core
P = 128
DCH = D // P  # 8 contraction chunks
NEG = -10000.0
WS = 64.0  # fp8 weight upscale (keeps quanta out of e4m3 subnormals)
VS = 2.0   # v / ctx storage scale

TRACE = False
_cache = {}


def build_program(S, apply_ln_affine, qk_lo=False, v_lo=False):
    """S: tuple of NB block widths (each >=64, mult of 4)."""
    S = list(S)
    off = np.concatenate([[0], np.cumsum(S)]).astype(int)  # block offsets
    T = int(off[-1])
    MT = (T + P - 1) // P  # 128-token tiles for S3
    QUADS = [(0, 4), (4, 8)]  # block groups for S2 merging
    CWMAX = max(sum(S[b0:b1]) for b0, b1 in QUADS)

    def pack_banks(sizes, bank=512, cap=1024):
        """Greedy offsets so no region crosses a 512-float psum bank."""
        offs, o = [], 0
        for s in sizes:
            if o // bank != (o + s - 1) // bank:
                o = (o // bank + 1) * bank
            offs.append(o)
            o += s
        assert o <= cap, f"psum pack overflow: {sizes}"
        return offs, o

    FWMAX = max(pack_banks([S[b] for b in range(b0, b1)
                            for _ in range(2)])[1] for b0, b1 in QUADS)
    TH0 = min(512, T)

    nc = bacc.Bacc("TRN2", target_bir_lowering=False, debug=False,
                   num_devices=NCORES)

    xT8 = nc.dram_tensor("xT8", [P, DCH, 2, T], FP8, kind="ExternalInput").ap()
    wqk8 = nc.dram_tensor("wqk8", [2 * DCH, P, DCH * P], FP8,
                          kind="ExternalInput").ap()
    wv8 = nc.dram_tensor("wv8", [2, P, DCH, 512], FP8,
                         kind="ExternalInput").ap()
    wo8 = nc.dram_tensor("wo8", [P, DCH, D], FP8, kind="ExternalInput").ap()
    ident8 = nc.dram_tensor("ident8", [P, 2, P], FP8, kind="ExternalInput").ap()
    bqk = nc.dram_tensor("bqk", [P, 2 * DCH], F32, kind="ExternalInput").ap()
    bo2 = nc.dram_tensor("bo2", [1, 2, D], FP8, kind="ExternalInput").ap()
    pmask = nc.dram_tensor("pmask", [1, NB, P], BF16, kind="ExternalInput").ap()
    lnw = nc.dram_tensor("lnw", [D], F32, kind="ExternalInput").ap()
    lnb = nc.dram_tensor("lnb", [D], F32, kind="ExternalInput").ap()
    y = nc.dram_tensor("y", [T, D], F32, kind="ExternalOutput").ap()

    DR = mybir.MatmulPerfMode.DoubleRow
    EXP = mybir.ActivationFunctionType.Exp
    CPY = mybir.ActivationFunctionType.Copy
    IDN = mybir.ActivationFunctionType.Identity
    SQR = mybir.ActivationFunctionType.Square
    SQRT = mybir.ActivationFunctionType.Sqrt

    with tile.TileContext(nc) as tc, ExitStack() as es:
        es.enter_context(nc.allow_low_precision(
            reason="fp8 ctx / bf16 scratch are intentional"))
        singles = es.enter_context(tc.tile_pool(name="singles", bufs=1))
        qkpool = es.enter_context(tc.tile_pool(name="qkpool", bufs=16))
        vpool = es.enter_context(tc.tile_pool(name="vpool", bufs=NB))
        ctxpool = es.enter_context(tc.tile_pool(name="ctxpool", bufs=1))
        exmpool = es.enter_context(tc.tile_pool(name="exmpool", bufs=3))
        sumpool = es.enter_context(tc.tile_pool(name="sumpool", bufs=2))
        rbpool = es.enter_context(tc.tile_pool(name="rbpool", bufs=3))
        s3pool = es.enter_context(tc.tile_pool(name="s3pool", bufs=4))
        # y_t tiles are per-m tags (MT live at once); yo rotates via bufs
        smallp = es.enter_context(tc.tile_pool(name="smallp", bufs=6))
        wpool = es.enter_context(tc.tile_pool(name="wpool", bufs=3))
        wvpool = es.enter_context(tc.tile_pool(name="wvpool", bufs=1))
        ps2 = es.enter_context(tc.tile_pool(name="ps2", bufs=3, space="PSUM"))
        ps1 = es.enter_context(tc.tile_pool(name="ps1", bufs=2, space="PSUM"))

        # --- resident inputs / constants (DMA order = need order) ---
        xT_sb = singles.tile([P, DCH, 2, T], FP8, tag="xT")
        nc.sync.dma_start(out=xT_sb[:, :, 0, :],
                          in_=xT8[:, :, 0, :])
        wo_sb = singles.tile([P, DCH, D], FP8, tag="wo")
        id_sb = singles.tile([P, 2, P], FP8, tag="ident")
        bqk_sb = singles.tile([P, 2 * DCH], F32, tag="bqk")
        nc.sync.dma_start(out=bqk_sb, in_=bqk)
        bo_sb = singles.tile([1, 2, D], FP8, tag="bo2")
        nc.sync.dma_start(out=bo_sb, in_=bo2)
        pm_sb = singles.tile([1, NB, P], BF16, tag="pmask")
        nc.sync.dma_start(out=pm_sb, in_=pmask)
        neg_sb = singles.tile([1, P], BF16, tag="negrow")
        nc.vector.memset(neg_sb, NEG)
        ones_sb = singles.tile([1, 2, P], FP8, tag="onesrow")
        nc.vector.memset(ones_sb[:, 0, :], 1.0)
        nc.vector.memset(ones_sb[:, 1, :], 0.0)
        eps_sb = singles.tile([P, 1], F32, tag="eps")
        nc.vector.memset(eps_sb, 1e-5)
        if apply_ln_affine:
            lnw_sb = singles.tile([P, D], F32, tag="lnw")
            nc.gpsimd.dma_start(out=lnw_sb, in_=bass.AP(
                tensor=lnw.tensor, offset=lnw.offset, ap=[[0, P], *lnw.ap]))
            lnb_sb = singles.tile([P, D], F32, tag="lnb")
            nc.gpsimd.dma_start(out=lnb_sb, in_=bass.AP(
                tensor=lnb.tensor, offset=lnb.offset, ap=[[0, P], *lnb.ap]))

        qkT_sb = [qkpool.tile([P, T], BF16, tag="qkT", name=f"qkT{i}")
                  for i in range(2 * DCH)]
        v_sb = [vpool.tile([S[b], D], BF16, tag=f"v{b}", name=f"v{b}")
                for b in range(NB)]
        ctx_sb = ctxpool.tile([P, DCH, T], FP8, tag="ctx8")

        THS = [(0, TH0)] + ([(TH0, T)] if T > TH0 else [])

        def emit_qk(wi):
            """q or k chunk wi (0-7 q, 8-15 k) -> qkT_sb[wi] bf16 [128, T]."""
            wt = wpool.tile([P, DCH, P], FP8, tag="wt", name=f"wt{wi}")
            nc.sync.dma_start(out=wt.rearrange("p c w -> p (c w)"),
                              in_=wqk8[wi])
            ps = ps2.tile([P, 1024], F32, tag="ps", name=f"qk{wi}")
            lo, hi = (0, 2) if qk_lo else (0, 1)
            for t0, t1 in THS:
                first = True
                for c in range(0, DCH, 2):
                    for li in range(lo, hi):
                        nc.tensor.matmul(
                            ps[:, t0:t1],
                            wt[:, c:c + 2, :],
                            xT_sb[:, c:c + 2, li, t0:t1],
                            start=first, stop=(c == DCH - 2 and li == hi - 1),
                            perf_mode=DR)
                        first = False
            qsc = (1.0 / np.sqrt(HD) if wi < DCH else 1.0) / WS
            nc.scalar.activation(out=qkT_sb[wi], in_=ps[:, 0:T], func=IDN,
                                 bias=bqk_sb[:, wi:wi + 1], scale=qsc)

        def emit_v(b, oh, wvt, eng):
            """v half oh for block b -> v_sb[b][:, oh*512:] (bias folded)."""
            bs = slice(int(off[b]), int(off[b + 1]))
            oc = slice(oh * 512, (oh + 1) * 512)
            ps = ps1.tile([P, 512], F32, tag="cx", name=f"v{b}_{oh}")
            lo, hi = (0, 2) if v_lo else (0, 1)
            first = True
            for c in range(0, DCH, 2):
                for li in range(lo, hi):
                    nc.tensor.matmul(
                        ps[0:S[b], :],
                        xT_sb[:, c:c + 2, li, bs],
                        wvt[:, c:c + 2, :],
                        start=first, stop=(c == DCH - 2 and li == hi - 1),
                        perf_mode=DR)
                    first = False
            if eng == "act":
                nc.scalar.activation(out=v_sb[b][:, oc], in_=ps[0:S[b], :],
                                     func=CPY, scale=VS / WS)
            else:
                nc.vector.tensor_scalar(
                    out=v_sb[b][:, oc], in0=ps[0:S[b], :], scalar1=VS / WS,
                    scalar2=None, op0=mybir.AluOpType.mult)

        def emit_s2(hp, quad):
            """head-pair hp, blocks quad[0]:quad[1] (4 blocks)."""
            b0, b1 = quad
            qa, ka = qkT_sb[hp], qkT_sb[DCH + hp]
            # per-block score regions of width 2*S_b, packed to psum banks
            roffs, FW = pack_banks([2 * S[b] for b in range(b0, b1)])
            qoff = {(b, j): roffs[b - b0] + j * S[b]
                    for b in range(b0, b1) for j in range(2)}
            kpmax = max(S[b0:b1])
            sc = ps2.tile([P, 1024], F32, tag="ps", name=f"sc{hp}_{b0}")
            for b in range(b0, b1):
                bs = slice(int(off[b]), int(off[b + 1]))
                for j in range(2):
                    rows = slice(j * HD, (j + 1) * HD)
                    reg = slice(qoff[b, j], qoff[b, j] + S[b])
                    nc.tensor.matmul(sc[0:S[b], reg], ka[rows, bs],
                                     qa[rows, bs], start=True, stop=False)
                    nc.tensor.matmul(sc[0:S[b], reg],
                                     pm_sb[:, b, 0:S[b]], neg_sb[:, 0:S[b]],
                                     start=False, stop=True)
            exm = exmpool.tile([P, FWMAX], BF16, tag="exm")
            nc.scalar.activation(out=exm[0:kpmax, 0:FW], in_=sc[0:kpmax, 0:FW],
                                 func=EXP)
            # row sums over keys (partitions) -> all partitions hold sum
            sums = sumpool.tile([P, 2, CWMAX], BF16, tag="sums")
            coff = [0] * NB  # free offsets of S_b regions in cx psum
            o = 0
            for b in range(b0, b1):
                coff[b] = o
                o += S[b]
            CW = o
            for b in range(b0, b1):
                nc.gpsimd.partition_all_reduce(
                    sums[0:S[b], :, coff[b]:coff[b] + S[b]],
                    exm[0:S[b], qoff[b, 0]:qoff[b, 0] + 2 * S[b]].rearrange(
                        "k (j q) -> k j q", j=2),
                    channels=S[b], reduce_op=bass_isa.ReduceOp.add)
            rb = rbpool.tile([P, CWMAX], BF16, tag="rb")
            nc.vector.reciprocal(out=rb[0:HD, 0:CW], in_=sums[0:HD, 0, 0:CW])
            nc.vector.reciprocal(out=rb[HD:P, 0:CW], in_=sums[0:HD, 1, 0:CW])
            cx = ps1.tile([P, 512], F32, tag="cx", name=f"cx{hp}_{b0}")
            for b in range(b0, b1):
                for j in range(2):
                    h = 2 * hp + j
                    reg = slice(qoff[b, j], qoff[b, j] + S[b])
                    nc.tensor.matmul(
                        cx[j * HD:(j + 1) * HD, coff[b]:coff[b] + S[b]],
                        v_sb[b][:, h * HD:(h + 1) * HD], exm[0:S[b], reg],
                        start=True, stop=True,
                        tile_position=(0, j * HD))
            ts = slice(int(off[b0]), int(off[b1]))
            nc.vector.tensor_mul(out=ctx_sb[:, hp, ts], in0=cx[:, 0:CW],
                                 in1=rb[:, 0:CW])

        y_ts = [singles.tile([P, D], BF16, tag=f"y{m}", name=f"y_t{m}")
                for m in range(MT)]
        sys_ = [singles.tile([P, 4], F32, tag=f"sy{m}", name=f"sy{m}")
                for m in range(MT)]

        def emit_s3a(m):
            """out_proj + residual + bias + stat sums for token tile m."""
            pt = min(P, T - m * P)
            mc = slice(m * P, m * P + pt)
            ps = ps2.tile([P, 1024], F32, tag="ps", name=f"s3_{m}")
            for oh in range(2):
                oc = slice(oh * 512, (oh + 1) * 512)
                for c in range(0, DCH, 2):
                    nc.tensor.matmul(ps[0:pt, oc],
                                     ctx_sb[:, c:c + 2, mc],
                                     wo_sb[:, c:c + 2, oc],
                                     start=(c == 0), stop=False,
                                     perf_mode=DR)
                # residual: += x (hi+lo) for the 4 ident chunks of this half
                for c in range(oh * 4, oh * 4 + 4):
                    nc.tensor.matmul(ps[0:pt, c * P:(c + 1) * P],
                                     xT_sb[:, c, :, mc], id_sb,
                                     start=False, stop=False, perf_mode=DR)
                # bias row (DR: row0 = ones x bo2, row1 = zeros)
                nc.tensor.matmul(ps[0:pt, oc], ones_sb[:, :, 0:pt],
                                 bo_sb[:, :, oc], start=False, stop=True,
                                 perf_mode=DR)
            y_t, sy = y_ts[m], sys_[m]
            nc.scalar.activation(out=y_t[0:pt], in_=ps[0:pt], func=CPY,
                                 scale=1.0 / (WS * VS),
                                 accum_out=sy[0:pt, 0:1])
            scr = singles.tile([P, D], BF16, tag="scr")
            nc.vector.scalar_tensor_tensor(
                out=scr[0:pt], in0=y_t[0:pt], scalar=0.0, in1=y_t[0:pt],
                op0=mybir.AluOpType.add, op1=mybir.AluOpType.mult,
                accum_out=sy[0:pt, 1:2])

        def emit_s3b(m):
            """LayerNorm apply + store for token tile m."""
            pt = min(P, T - m * P)
            mc = slice(m * P, m * P + pt)
            y_t, sy = y_ts[m], sys_[m]
            # mu = sy/D ; var = sq/D - mu^2 ; rstd = 1/sqrt(var+eps)
            mv = smallp.tile([P, 4], F32, tag="mv")
            nc.scalar.activation(out=mv[0:pt, 0:2], in_=sy[0:pt, 0:2],
                                 func=CPY, scale=1.0 / D)
            nc.vector.tensor_tensor(out=mv[0:pt, 2:3], in0=mv[0:pt, 0:1],
                                    in1=mv[0:pt, 0:1], op=mybir.AluOpType.mult)
            nc.vector.tensor_tensor(out=mv[0:pt, 3:4], in0=mv[0:pt, 1:2],
                                    in1=mv[0:pt, 2:3],
                                    op=mybir.AluOpType.subtract)
            rstd = smallp.tile([P, 2], F32, tag="rstd")
            nc.scalar.activation(out=rstd[0:pt, 1:2], in_=mv[0:pt, 3:4],
                                 func=SQRT, bias=eps_sb[0:pt], scale=1.0)
            nc.vector.reciprocal(out=rstd[0:pt, 1:2], in_=rstd[0:pt, 1:2])
            for h in range(2):
                hc = slice(h * 512, (h + 1) * 512)
                yo = s3pool.tile([P, 512], F32, tag="yo")
                nc.vector.tensor_scalar(
                    out=yo[0:pt], in0=y_t[0:pt, hc], scalar1=mv[0:pt, 0:1],
                    scalar2=rstd[0:pt, 1:2], op0=mybir.AluOpType.subtract,
                    op1=mybir.AluOpType.mult)
                if apply_ln_affine:
                    nc.vector.tensor_mul(out=yo[0:pt], in0=yo[0:pt],
                                         in1=lnw_sb[0:pt, hc])
                    nc.vector.tensor_add(out=yo[0:pt], in0=yo[0:pt],
                                         in1=lnb_sb[0:pt, hc])
                nc.sync.dma_start(out=y[mc, hc], in_=yo[0:pt])

        # --- emission schedule: interleave S1 qk/v with S2 so the vector
        # engines start early; S3 after all S2 (ctx complete). ---
        for wi in (0, 8):
            emit_qk(wi)
        wv0 = wvpool.tile([P, DCH, 512], FP8, tag="wv", name="wv0")
        nc.sync.dma_start(out=wv0, in_=wv8[0])
        for b in range(NB):
            emit_v(b, 0, wv0, "act" if b % 2 else "dve")
        emit_s2(0, QUADS[0])
        emit_qk(1)
        emit_qk(9)
        emit_qk(2)
        emit_qk(10)
        emit_s2(1, QUADS[0])
        emit_qk(3)
        emit_qk(11)
        emit_s2(2, QUADS[0])
        emit_qk(4)
        emit_qk(12)
        emit_s2(3, QUADS[0])
        wv1 = wvpool.tile([P, DCH, 512], FP8, tag="wv", name="wv1")
        nc.sync.dma_start(out=wv1, in_=wv8[1])
        nc.sync.dma_start(out=xT_sb[:, :, 1, :], in_=xT8[:, :, 1, :])
        nc.sync.dma_start(out=wo_sb, in_=wo8)
        nc.sync.dma_start(out=id_sb, in_=ident8)
        for b in range(NB):
            emit_v(b, 1, wv1, "act" if b % 2 else "dve")
        emit_qk(5)
        emit_qk(13)
        emit_s2(4, QUADS[0])
        emit_qk(6)
        emit_qk(14)
        emit_s2(5, QUADS[0])
        emit_qk(7)
        emit_qk(15)
        emit_s2(6, QUADS[0])
        emit_s2(7, QUADS[0])
        emit_s2(0, QUADS[1])
        emit_s3a(0)
        emit_s2(1, QUADS[1])
        emit_s3a(1)
        for hp in range(2, 8):
            emit_s2(hp, QUADS[1])
        emit_s3b(0)
        emit_s3b(1)
        for m in range(2, MT):
            emit_s3a(m)
        for m in range(2, MT):
            emit_s3b(m)

    nc.compile()
    return nc


def _numpy_fallback(x, cluster_id, in_proj_w, in_proj_b, out_proj_w,
                    out_proj_b, ln_w, ln_b, num_heads):
    n, d = x.shape
    hd = d // num_heads
    x64 = x.astype(np.float64)
    qkv = x64 @ in_proj_w.T.astype(np.float64) + in_proj_b
    q, k, v = np.split(qkv, 3, axis=-1)
    q = q.reshape(n, num_heads, hd)
    k = k.reshape(n, num_heads, hd)
    v = v.reshape(n, num_heads, hd)
    valid = cluster_id >= 0
    allow = (cluster_id[:, None] == cluster_id[None, :]) & valid[:, None] & valid[None, :]
    scores = np.einsum("nhd,mhd->hnm", q, k) / np.sqrt(hd)
    scores = np.where(allow[None], scores, np.finfo(np.float32).min)
    scores -= scores.max(axis=-1, keepdims=True)
    e = np.exp(scores)
    attn = e / e.sum(axis=-1, keepdims=True)
    ctx = np.einsum("hnm,mhd->nhd", attn, v).reshape(n, d)
    yv = x64 + ctx @ out_proj_w.T.astype(np.float64) + out_proj_b
    mu = yv.mean(-1, keepdims=True)
    var = ((yv - mu) ** 2).mean(-1, keepdims=True)
    out = (yv - mu) / np.sqrt(var + 1e-5) * ln_w + ln_b
    return out.astype(np.float32)


def _hi_lo(a):
    hi = a.astype(NFP8)
    lo = (a - hi.astype(np.float32)).astype(NFP8)
    return hi, lo


def kernel(x, cluster_id, in_proj_w, in_proj_b, out_proj_w, out_proj_b,
           ln_w, ln_b, num_heads):
    x = np.asarray(x, dtype=np.float32)
    cid = np.asarray(cluster_id).astype(np.int64)
    in_proj_w = np.asarray(in_proj_w, dtype=np.float32)
    in_proj_b = np.asarray(in_proj_b, dtype=np.float32)
    out_proj_w = np.asarray(out_proj_w, dtype=np.float32)
    out_proj_b = np.asarray(out_proj_b, dtype=np.float32)
    ln_w = np.asarray(ln_w, dtype=np.float32)
    ln_b = np.asarray(ln_b, dtype=np.float32)
    nh = int(np.asarray(num_heads))

    counts = np.bincount(cid, minlength=NCLUST) if cid.size and cid.min() >= 0 else None
    if (x.shape != (N, D) or nh != H or counts is None
            or cid.max() >= NCLUST or counts.max() > 120):
        return _numpy_fallback(x, cid, in_proj_w, in_proj_b, out_proj_w,
                               out_proj_b, ln_w, ln_b, nh)

    # --- cluster -> (core, block) assignment: sort clusters by size desc,
    # rank 8j+c -> core c block j; S_j = max(ceil4(max size in group), 64).
    order_cl = np.argsort(-counts, kind="stable")
    assign = np.empty((NCORES, NB), dtype=np.int64)
    S = []
    for j in range(NB):
        grp = order_cl[j * NCORES:(j + 1) * NCORES]
        assign[:, j] = grp
        S.append(max(int(-(-int(counts[grp].max()) // 4) * 4), 64))
    S = tuple(S)
    off = np.concatenate([[0], np.cumsum(S)]).astype(int)
    T = int(off[-1])

    ln_trivial = bool(np.all(ln_w == 1.0) and np.all(ln_b == 0.0))
    key = (S, not ln_trivial)
    if key not in _cache:
        _cache[key] = build_program(S, not ln_trivial)
    nc = _cache[key]

    # --- shared (replicated) weight prep ---
    scale = 1.0 / np.sqrt(HD)
    wqk_t = np.ascontiguousarray(in_proj_w[:2 * D].T) * WS  # [D, 2D]
    bqk_f = in_proj_b[:2 * D].copy()
    bqk_f[:D] *= scale
    wv_t = np.ascontiguousarray(in_proj_w[2 * D:].T) * WS  # [D, D]
    bv = in_proj_b[2 * D:]
    wo_t = np.ascontiguousarray(out_proj_w.T)  # [D, D]
    bo2 = ((out_proj_b + bv @ wo_t) * (WS * VS)).astype(np.float32)

    ident = np.zeros((P, 2, P), dtype=NFP8)
    ii = np.arange(P)
    ident[ii, 0, ii] = WS * VS
    ident[ii, 1, ii] = WS * VS

    common = dict(
        wqk8=np.ascontiguousarray(
            wqk_t.reshape(DCH, P, 2 * DCH, P).transpose(2, 1, 0, 3)).astype(NFP8),
        wv8=np.ascontiguousarray(
            wv_t.reshape(DCH, P, 2, 512).transpose(2, 1, 0, 3)).astype(NFP8),
        wo8=np.ascontiguousarray(
            (wo_t * WS).reshape(DCH, P, D).transpose(1, 0, 2)).astype(NFP8),
        ident8=ident,
        bqk=np.ascontiguousarray(
            bqk_f.reshape(2 * DCH, P).T).astype(np.float32),
        bo2=np.stack([bo2, np.zeros_like(bo2)])[None].astype(NFP8),
        lnw=ln_w, lnb=ln_b)

    # token order per cluster
    sort_tok = np.argsort(cid, kind="stable")
    cl_start = np.concatenate([[0], np.cumsum(counts)]).astype(int)

    in_maps = []
    slot_tok = []
    for core in range(NCORES):
        xp = np.zeros((T, D), dtype=np.float32)
        pm = np.ones((NB, P), dtype=np.float32)
        slots = []
        toks = []
        for j in range(NB):
            cl = int(assign[core, j])
            nk = int(counts[cl])
            tk = sort_tok[cl_start[cl]:cl_start[cl] + nk]
            sl = np.arange(int(off[j]), int(off[j]) + nk)
            xp[sl] = x[tk]
            pm[j, :nk] = 0.0
            slots.append(sl)
            toks.append(tk)
        im = dict(common)
        xT = np.ascontiguousarray(xp.T)  # [D, T]
        hi, lo = _hi_lo(xT)
        x8 = np.empty((P, DCH, 2, T), dtype=NFP8)
        x8[:, :, 0, :] = hi.reshape(DCH, P, T).transpose(1, 0, 2)
        x8[:, :, 1, :] = lo.reshape(DCH, P, T).transpose(1, 0, 2)
        im["xT8"] = x8
        im["pmask"] = pm[None].astype(NBF16)
        in_maps.append(im)
        slot_tok.append((np.concatenate(slots), np.concatenate(toks)))

    res = run_bass_kernel_spmd(nc, in_maps, core_ids=list(range(NCORES)),
                               trace=TRACE)
    kernel.last_results = res

    out = np.empty((N, D), dtype=np.float32)
    for core in range(NCORES):
        slots, toks = slot_tok[core]
        out[toks] = res.results[core]["y"][slots]
    return out


# revision 63
# speedup vs baseline: 1.0168x; 1.0168x over previous
"""MaskClusterAttention Trainium2 kernel (fp8 DoubleRow redesign).

Sparse attention: tokens attend only within their cluster (64 clusters,
~64 tokens each).  Host sorts clusters by size and assigns rank
8j+c -> (core c, block j), so all 8 cores share one SPMD program with
per-block compile-time widths S_j = max size in rank-group j (>=64,
mult of 4).  Per-core padding is ~9% instead of the 50% a fixed 96-pad
costs.

Per core (T = sum S_j tokens):
  S1: q,k projections emitted transposed [128, T] via fp8e4 DoubleRow
      matmuls (2x128 contraction per instruction, 0.5 cyc/row); v per
      block in [S_j, 1024] layout.  x is staged as interleaved fp8
      hi/lo pairs [128, 8, 2, T] so single-fp8 (hi) and compensated
      (hi+lo) passes share one layout.  q scaled by 1/sqrt(hd) and in_proj
      bias applied during the PSUM->SBUF copy (Act, per-partition bias).
  S2: per (head-pair, block): scoresT [S_j, 2, S_j] in PSUM; pad-key
      masking is a rank-1 matmul (padflag x -1e4 row) accumulated into
      the same PSUM group, so exp needs no bias and merges 4 blocks per
      Act op.  Row sums via gpsimd partition_all_reduce, reciprocal
      (DVE, bf16), normalize-multiply fused with the fp8 downcast of
      ctxT.
  S3: out_proj via fp8 DoubleRow from ctxT8; residual x and bias bo'
      (bo + bv@Wo, v-bias folded out) enter the same PSUM via identity /
      ones rank-1 matmuls.  LayerNorm stats come free from Act accum_out
      (sum y, sum y^2), apply via one tensor_scalar.  No collectives.

Host scatters per-core outputs back through the slot map.
"""

from contextlib import ExitStack

import ml_dtypes
import numpy as np

import concourse.bass as bass
import concourse.bass_isa as bass_isa
import concourse.mybir as mybir
import concourse.tile as tile
from concourse import bacc
from concourse.bass_utils import run_bass_kernel_spmd

F32 = mybir.dt.float32
BF16 = mybir.dt.bfloat16
FP8 = mybir.dt.float8e4
NFP8 = ml_dtypes.float8_e4m3
NBF16 = ml_dtypes.bfloat16

N, D, H, HD, NCLUST, NCORES = 4096, 1024, 16, 64, 64, 8
NB = NCLUST // NCORES  # blocks (clusters) per core
P = 128
DCH = D // P  # 8 contraction chunks
NEG = -10000.0
WS = 64.0  # fp8 weight upscale (keeps quanta out of e4m3 subnormals)
VS = 2.0   # v / ctx storage scale

TRACE = False
_cache = {}


def build_program(S, apply_ln_affine, qk_lo=False, v_lo=False):
    """S: tuple of NB block widths (each >=64, mult of 4)."""
    S = list(S)
    off = np.concatenate([[0], np.cumsum(S)]).astype(int)  # block offsets
    T = int(off[-1])
    MT = (T + P - 1) // P  # 128-token tiles for S3
    QUADS = [(0, 4), (4, 8)]  # block groups for S2 merging
    CWMAX = max(sum(S[b0:b1]) for b0, b1 in QUADS)

    def pack_banks(sizes, bank=512, cap=1024):
        """Greedy offsets so no region crosses a 512-float psum bank."""
        offs, o = [], 0
        for s in sizes:
            if o // bank != (o + s - 1) // bank:
                o = (o // bank + 1) * bank
            offs.append(o)
            o += s
        assert o <= cap, f"psum pack overflow: {sizes}"
        return offs, o

    FWMAX = max(pack_banks([S[b] for b in range(b0, b1)
                            for _ in range(2)])[1] for b0, b1 in QUADS)
    TH0 = min(512, T)

    nc = bacc.Bacc("TRN2", target_bir_lowering=False, debug=False,
                   num_devices=NCORES)

    xT8 = nc.dram_tensor("xT8", [P, DCH, 2, T], FP8, kind="ExternalInput").ap()
    wqk8 = nc.dram_tensor("wqk8", [2 * DCH, P, DCH * P], FP8,
                          kind="ExternalInput").ap()
    wv8 = nc.dram_tensor("wv8", [2, P, DCH, 512], FP8,
                         kind="ExternalInput").ap()
    wo8 = nc.dram_tensor("wo8", [P, DCH, D], FP8, kind="ExternalInput").ap()
    ident8 = nc.dram_tensor("ident8", [P, 2, P], FP8, kind="ExternalInput").ap()
    bqk = nc.dram_tensor("bqk", [P, 2 * DCH], F32, kind="ExternalInput").ap()
    bo2 = nc.dram_tensor("bo2", [1, 2, D], FP8, kind="ExternalInput").ap()
    pmask = nc.dram_tensor("pmask", [1, NB, P], BF16, kind="ExternalInput").ap()
    lnw = nc.dram_tensor("lnw", [D], F32, kind="ExternalInput").ap()
    lnb = nc.dram_tensor("lnb", [D], F32, kind="ExternalInput").ap()
    y = nc.dram_tensor("y", [T, D], F32, kind="ExternalOutput").ap()

    DR = mybir.MatmulPerfMode.DoubleRow
    EXP = mybir.ActivationFunctionType.Exp
    CPY = mybir.ActivationFunctionType.Copy
    IDN = mybir.ActivationFunctionType.Identity
    SQR = mybir.ActivationFunctionType.Square
    SQRT = mybir.ActivationFunctionType.Sqrt

    with tile.TileContext(nc) as tc, ExitStack() as es:
        es.enter_context(nc.allow_low_precision(
            reason="fp8 ctx / bf16 scratch are intentional"))
        singles = es.enter_context(tc.tile_pool(name="singles", bufs=1))
        qkpool = es.enter_context(tc.tile_pool(name="qkpool", bufs=16))
        vpool = es.enter_context(tc.tile_pool(name="vpool", bufs=NB))
        ctxpool = es.enter_context(tc.tile_pool(name="ctxpool", bufs=1))
        exmpool = es.enter_context(tc.tile_pool(name="exmpool", bufs=3))
        sumpool = es.enter_context(tc.tile_pool(name="sumpool", bufs=2))
        rbpool = es.enter_context(tc.tile_pool(name="rbpool", bufs=3))
        s3pool = es.enter_context(tc.tile_pool(name="s3pool", bufs=4))
        # y_t tiles are per-m tags (MT live at once); yo rotates via bufs
        smallp = es.enter_context(tc.tile_pool(name="smallp", bufs=6))
        wpool = es.enter_context(tc.tile_pool(name="wpool", bufs=3))
        wvpool = es.enter_context(tc.tile_pool(name="wvpool", bufs=1))
        ps2 = es.enter_context(tc.tile_pool(name="ps2", bufs=3, space="PSUM"))
        ps1 = es.enter_context(tc.tile_pool(name="ps1", bufs=2, space="PSUM"))

        # --- resident inputs / constants (DMA order = need order) ---
        xT_sb = singles.tile([P, DCH, 2, T], FP8, tag="xT")
        nc.sync.dma_start(out=xT_sb[:, :, 0, :],
                          in_=xT8[:, :, 0, :])
        wo_sb = singles.tile([P, DCH, D], FP8, tag="wo")
        id_sb = singles.tile([P, 2, P], FP8, tag="ident")
        bqk_sb = singles.tile([P, 2 * DCH], F32, tag="bqk")
        nc.sync.dma_start(out=bqk_sb, in_=bqk)
        bo_sb = singles.tile([1, 2, D], FP8, tag="bo2")
        nc.sync.dma_start(out=bo_sb, in_=bo2)
        pm_sb = singles.tile([1, NB, P], BF16, tag="pmask")
        nc.sync.dma_start(out=pm_sb, in_=pmask)
        neg_sb = singles.tile([1, P], BF16, tag="negrow")
        nc.vector.memset(neg_sb, NEG)
        ones_sb = singles.tile([1, 2, P], FP8, tag="onesrow")
        nc.vector.memset(ones_sb[:, 0, :], 1.0)
        nc.vector.memset(ones_sb[:, 1, :], 0.0)
        eps_sb = singles.tile([P, 1], F32, tag="eps")
        nc.vector.memset(eps_sb, 1e-5)
        if apply_ln_affine:
            lnw_sb = singles.tile([P, D], F32, tag="lnw")
            nc.gpsimd.dma_start(out=lnw_sb, in_=bass.AP(
                tensor=lnw.tensor, offset=lnw.offset, ap=[[0, P], *lnw.ap]))
            lnb_sb = singles.tile([P, D], F32, tag="lnb")
            nc.gpsimd.dma_start(out=lnb_sb, in_=bass.AP(
                tensor=lnb.tensor, offset=lnb.offset, ap=[[0, P], *lnb.ap]))

        qkT_sb = [qkpool.tile([P, T], BF16, tag="qkT", name=f"qkT{i}")
                  for i in range(2 * DCH)]
        v_sb = [vpool.tile([S[b], D], BF16, tag=f"v{b}", name=f"v{b}")
                for b in range(NB)]
        ctx_sb = ctxpool.tile([P, DCH, T], FP8, tag="ctx8")

        THS = [(0, TH0)] + ([(TH0, T)] if T > TH0 else [])

        def emit_qk(wi):
            """q or k chunk wi (0-7 q, 8-15 k) -> qkT_sb[wi] bf16 [128, T]."""
            wt = wpool.tile([P, DCH, P], FP8, tag="wt", name=f"wt{wi}")
            nc.sync.dma_start(out=wt.rearrange("p c w -> p (c w)"),
                              in_=wqk8[wi])
            ps = ps2.tile([P, 1024], F32, tag="ps", name=f"qk{wi}")
            lo, hi = (0, 2) if qk_lo else (0, 1)
            for t0, t1 in THS:
                first = True
                for c in range(0, DCH, 2):
                    for li in range(lo, hi):
                        nc.tensor.matmul(
                            ps[:, t0:t1],
                            wt[:, c:c + 2, :],
                            xT_sb[:, c:c + 2, li, t0:t1],
                            start=first, stop=(c == DCH - 2 and li == hi - 1),
                            perf_mode=DR)
                        first = False
            qsc = (1.0 / np.sqrt(HD) if wi < DCH else 1.0) / WS
            nc.scalar.activation(out=qkT_sb[wi], in_=ps[:, 0:T], func=IDN,
                                 bias=bqk_sb[:, wi:wi + 1], scale=qsc)

        def emit_v(b, oh, wvt, eng):
            """v half oh for block b -> v_sb[b][:, oh*512:] (bias folded)."""
            bs = slice(int(off[b]), int(off[b + 1]))
            oc = slice(oh * 512, (oh + 1) * 512)
            ps = ps1.tile([P, 512], F32, tag="cx", name=f"v{b}_{oh}")
            lo, hi = (0, 2) if v_lo else (0, 1)
            first = True
            for c in range(0, DCH, 2):
                for li in range(lo, hi):
                    nc.tensor.matmul(
                        ps[0:S[b], :],
                        xT_sb[:, c:c + 2, li, bs],
                        wvt[:, c:c + 2, :],
                        start=first, stop=(c == DCH - 2 and li == hi - 1),
                        perf_mode=DR)
                    first = False
            if eng == "act":
                nc.scalar.activation(out=v_sb[b][:, oc], in_=ps[0:S[b], :],
                                     func=CPY, scale=VS / WS)
            else:
                nc.vector.tensor_scalar(
                    out=v_sb[b][:, oc], in0=ps[0:S[b], :], scalar1=VS / WS,
                    scalar2=None, op0=mybir.AluOpType.mult)

        def emit_s2(hp, quad):
            """head-pair hp, blocks quad[0]:quad[1] (4 blocks)."""
            b0, b1 = quad
            qa, ka = qkT_sb[hp], qkT_sb[DCH + hp]
            # per-block score regions of width 2*S_b, packed to psum banks
            roffs, FW = pack_banks([2 * S[b] for b in range(b0, b1)])
            qoff = {(b, j): roffs[b - b0] + j * S[b]
                    for b in range(b0, b1) for j in range(2)}
            kpmax = max(S[b0:b1])
            sc = ps2.tile([P, 1024], F32, tag="ps", name=f"sc{hp}_{b0}")
            for b in range(b0, b1):
                bs = slice(int(off[b]), int(off[b + 1]))
                for j in range(2):
                    rows = slice(j * HD, (j + 1) * HD)
                    reg = slice(qoff[b, j], qoff[b, j] + S[b])
                    nc.tensor.matmul(sc[0:S[b], reg], ka[rows, bs],
                                     qa[rows, bs], start=True, stop=False)
                    nc.tensor.matmul(sc[0:S[b], reg],
                                     pm_sb[:, b, 0:S[b]], neg_sb[:, 0:S[b]],
                                     start=False, stop=True)
            exm = exmpool.tile([P, FWMAX], BF16, tag="exm")
            nc.scalar.activation(out=exm[0:kpmax, 0:FW], in_=sc[0:kpmax, 0:FW],
                                 func=EXP)
            # row sums over keys (partitions) -> all partitions hold sum
            sums = sumpool.tile([P, 2, CWMAX], BF16, tag="sums")
            coff = [0] * NB  # free offsets of S_b regions in cx psum
            o = 0
            for b in range(b0, b1):
                coff[b] = o
                o += S[b]
            CW = o
            for b in range(b0, b1):
                nc.gpsimd.partition_all_reduce(
                    sums[0:S[b], :, coff[b]:coff[b] + S[b]],
                    exm[0:S[b], qoff[b, 0]:qoff[b, 0] + 2 * S[b]].rearrange(
                        "k (j q) -> k j q", j=2),
                    channels=S[b], reduce_op=bass_isa.ReduceOp.add)
            rb = rbpool.tile([P, CWMAX], BF16, tag="rb")
            nc.vector.reciprocal(out=rb[0:HD, 0:CW], in_=sums[0:HD, 0, 0:CW])
            nc.vector.reciprocal(out=rb[HD:P, 0:CW], in_=sums[0:HD, 1, 0:CW])
            cx = ps1.tile([P, 512], F32, tag="cx", name=f"cx{hp}_{b0}")
            for b in range(b0, b1):
                for j in range(2):
                    h = 2 * hp + j
                    reg = slice(qoff[b, j], qoff[b, j] + S[b])
                    nc.tensor.matmul(
                        cx[j * HD:(j + 1) * HD, coff[b]:coff[b] + S[b]],
                        v_sb[b][:, h * HD:(h + 1) * HD], exm[0:S[b], reg],
                        start=True, stop=True,
                        tile_position=(0, j * HD))
            ts = slice(int(off[b0]), int(off[b1]))
            nc.vector.tensor_mul(out=ctx_sb[:, hp, ts], in0=cx[:, 0:CW],
                                 in1=rb[:, 0:CW])

        y_ts = [singles.tile([P, D], BF16, tag=f"y{m}", name=f"y_t{m}")
                for m in range(MT)]
        sys_ = [singles.tile([P, 4], F32, tag=f"sy{m}", name=f"sy{m}")
                for m in range(MT)]

        def emit_s3a(m):
            """out_proj + residual + bias + stat sums for token tile m."""
            pt = min(P, T - m * P)
            mc = slice(m * P, m * P + pt)
            ps = ps2.tile([P, 1024], F32, tag="ps", name=f"s3_{m}")
            for oh in range(2):
                oc = slice(oh * 512, (oh + 1) * 512)
                for c in range(0, DCH, 2):
                    nc.tensor.matmul(ps[0:pt, oc],
                                     ctx_sb[:, c:c + 2, mc],
                                     wo_sb[:, c:c + 2, oc],
                                     start=(c == 0), stop=False,
                                     perf_mode=DR)
                # residual: += x (hi+lo) for the 4 ident chunks of this half
                for c in range(oh * 4, oh * 4 + 4):
                    nc.tensor.matmul(ps[0:pt, c * P:(c + 1) * P],
                                     xT_sb[:, c, :, mc], id_sb,
                                     start=False, stop=False, perf_mode=DR)
                # bias row (DR: row0 = ones x bo2, row1 = zeros)
                nc.tensor.matmul(ps[0:pt, oc], ones_sb[:, :, 0:pt],
                                 bo_sb[:, :, oc], start=False, stop=True,
                                 perf_mode=DR)
            y_t, sy = y_ts[m], sys_[m]
            nc.scalar.activation(out=y_t[0:pt], in_=ps[0:pt], func=CPY,
                                 scale=1.0 / (WS * VS),
                                 accum_out=sy[0:pt, 0:1])
            scr = singles.tile([P, D], BF16, tag="scr")
            nc.vector.scalar_tensor_tensor(
                out=scr[0:pt], in0=y_t[0:pt], scalar=0.0, in1=y_t[0:pt],
                op0=mybir.AluOpType.add, op1=mybir.AluOpType.mult,
                accum_out=sy[0:pt, 1:2])

        def emit_s3b(m):
            """LayerNorm apply + store for token tile m."""
            pt = min(P, T - m * P)
            mc = slice(m * P, m * P + pt)
            y_t, sy = y_ts[m], sys_[m]
            # mu = sy/D ; var = sq/D - mu^2 ; rstd = 1/sqrt(var+eps)
            mv = smallp.tile([P, 4], F32, tag="mv")
            nc.scalar.activation(out=mv[0:pt, 0:2], in_=sy[0:pt, 0:2],
                                 func=CPY, scale=1.0 / D)
            nc.vector.tensor_tensor(out=mv[0:pt, 2:3], in0=mv[0:pt, 0:1],
                                    in1=mv[0:pt, 0:1], op=mybir.AluOpType.mult)
            nc.vector.tensor_tensor(out=mv[0:pt, 3:4], in0=mv[0:pt, 1:2],
                                    in1=mv[0:pt, 2:3],
                                    op=mybir.AluOpType.subtract)
            rstd = smallp.tile([P, 2], F32, tag="rstd")
            nc.scalar.activation(out=rstd[0:pt, 1:2], in_=mv[0:pt, 3:4],
                                 func=SQRT, bias=eps_sb[0:pt], scale=1.0)
            nc.vector.reciprocal(out=rstd[0:pt, 1:2], in_=rstd[0:pt, 1:2])
            for h in range(2):
                hc = slice(h * 512, (h + 1) * 512)
                yo = s3pool.tile([P, 512], F32, tag="yo")
                nc.vector.tensor_scalar(
                    out=yo[0:pt], in0=y_t[0:pt, hc], scalar1=mv[0:pt, 0:1],
                    scalar2=rstd[0:pt, 1:2], op0=mybir.AluOpType.subtract,
                    op1=mybir.AluOpType.mult)
                if apply_ln_affine:
                    nc.vector.tensor_mul(out=yo[0:pt], in0=yo[0:pt],
                                         in1=lnw_sb[0:pt, hc])
                    nc.vector.tensor_add(out=yo[0:pt], in0=yo[0:pt],
                                         in1=lnb_sb[0:pt, hc])
                nc.sync.dma_start(out=y[mc, hc], in_=yo[0:pt])

        # --- emission schedule: interleave S1 qk/v with S2 so the vector
        # engines start early; S3 after all S2 (ctx complete). ---
        for wi in (0, 8, 1, 9):
            emit_qk(wi)
        wv0 = wvpool.tile([P, DCH, 512], FP8, tag="wv", name="wv0")
        nc.sync.dma_start(out=wv0, in_=wv8[0])
        for b in range(NB):
            emit_v(b, 0, wv0, "act" if b % 2 else "dve")
        emit_s2(0, QUADS[0])
        emit_qk(2)
        emit_qk(10)
        emit_s2(1, QUADS[0])
        emit_qk(3)
        emit_qk(11)
        emit_s2(2, QUADS[0])
        emit_qk(4)
        emit_qk(12)
        emit_s2(3, QUADS[0])
        wv1 = wvpool.tile([P, DCH, 512], FP8, tag="wv", name="wv1")
        nc.sync.dma_start(out=wv1, in_=wv8[1])
        nc.sync.dma_start(out=xT_sb[:, :, 1, :], in_=xT8[:, :, 1, :])
        nc.sync.dma_start(out=wo_sb, in_=wo8)
        nc.sync.dma_start(out=id_sb, in_=ident8)
        for b in range(NB):
            emit_v(b, 1, wv1, "act" if b % 2 else "dve")
        emit_qk(5)
        emit_qk(13)
        emit_s2(4, QUADS[0])
        emit_qk(6)
        emit_qk(14)
        emit_s2(5, QUADS[0])
        emit_qk(7)
        emit_qk(15)
        emit_s2(6, QUADS[0])
        emit_s2(7, QUADS[0])
        emit_s2(0, QUADS[1])
        emit_s3a(0)
        emit_s2(1, QUADS[1])
        emit_s3a(1)
        for hp in range(2, 8):
            emit_s2(hp, QUADS[1])
        emit_s3b(0)
        emit_s3b(1)
        for m in range(2, MT):
            emit_s3a(m)
        for m in range(2, MT):
            emit_s3b(m)

    nc.compile()
    return nc


def _numpy_fallback(x, cluster_id, in_proj_w, in_proj_b, out_proj_w,
                    out_proj_b, ln_w, ln_b, num_heads):
    n, d = x.shape
    hd = d // num_heads
    x64 = x.astype(np.float64)
    qkv = x64 @ in_proj_w.T.astype(np.float64) + in_proj_b
    q, k, v = np.split(qkv, 3, axis=-1)
    q = q.reshape(n, num_heads, hd)
    k = k.reshape(n, num_heads, hd)
    v = v.reshape(n, num_heads, hd)
    valid = cluster_id >= 0
    allow = (cluster_id[:, None] == cluster_id[None, :]) & valid[:, None] & valid[None, :]
    scores = np.einsum("nhd,mhd->hnm", q, k) / np.sqrt(hd)
    scores = np.where(allow[None], scores, np.finfo(np.float32).min)
    scores -= scores.max(axis=-1, keepdims=True)
    e = np.exp(scores)
    attn = e / e.sum(axis=-1, keepdims=True)
    ctx = np.einsum("hnm,mhd->nhd", attn, v).reshape(n, d)
    yv = x64 + ctx @ out_proj_w.T.astype(np.float64) + out_proj_b
    mu = yv.mean(-1, keepdims=True)
    var = ((yv - mu) ** 2).mean(-1, keepdims=True)
    out = (yv - mu) / np.sqrt(var + 1e-5) * ln_w + ln_b
    return out.astype(np.float32)


def _hi_lo(a):
    hi = a.astype(NFP8)
    lo = (a - hi.astype(np.float32)).astype(NFP8)
    return hi, lo


def kernel(x, cluster_id, in_proj_w, in_proj_b, out_proj_w, out_proj_b,
           ln_w, ln_b, num_heads):
    x = np.asarray(x, dtype=np.float32)
    cid = np.asarray(cluster_id).astype(np.int64)
    in_proj_w = np.asarray(in_proj_w, dtype=np.float32)
    in_proj_b = np.asarray(in_proj_b, dtype=np.float32)
    out_proj_w = np.asarray(out_proj_w, dtype=np.float32)
    out_proj_b = np.asarray(out_proj_b, dtype=np.float32)
    ln_w = np.asarray(ln_w, dtype=np.float32)
    ln_b = np.asarray(ln_b, dtype=np.float32)
    nh = int(np.asarray(num_heads))

    counts = np.bincount(cid, minlength=NCLUST) if cid.size and cid.min() >= 0 else None
    if (x.shape != (N, D) or nh != H or counts is None
            or cid.max() >= NCLUST or counts.max() > 120):
        return _numpy_fallback(x, cid, in_proj_w, in_proj_b, out_proj_w,
                               out_proj_b, ln_w, ln_b, nh)

    # --- cluster -> (core, block) assignment: sort clusters by size desc,
    # rank 8j+c -> core c block j; S_j = max(ceil4(max size in group), 64).
    order_cl = np.argsort(-counts, kind="stable")
    assign = np.empty((NCORES, NB), dtype=np.int64)
    S = []
    for j in range(NB):
        grp = order_cl[j * NCORES:(j + 1) * NCORES]
        assign[:, j] = grp
        S.append(max(int(-(-int(counts[grp].max()) // 4) * 4), 64))
    S = tuple(S)
    off = np.concatenate([[0], np.cumsum(S)]).astype(int)
    T = int(off[-1])

    ln_trivial = bool(np.all(ln_w == 1.0) and np.all(ln_b == 0.0))
    key = (S, not ln_trivial)
    if key not in _cache:
        _cache[key] = build_program(S, not ln_trivial)
    nc = _cache[key]

    # --- shared (replicated) weight prep ---
    scale = 1.0 / np.sqrt(HD)
    wqk_t = np.ascontiguousarray(in_proj_w[:2 * D].T) * WS  # [D, 2D]
    bqk_f = in_proj_b[:2 * D].copy()
    bqk_f[:D] *= scale
    wv_t = np.ascontiguousarray(in_proj_w[2 * D:].T) * WS  # [D, D]
    bv = in_proj_b[2 * D:]
    wo_t = np.ascontiguousarray(out_proj_w.T)  # [D, D]
    bo2 = ((out_proj_b + bv @ wo_t) * (WS * VS)).astype(np.float32)

    ident = np.zeros((P, 2, P), dtype=NFP8)
    ii = np.arange(P)
    ident[ii, 0, ii] = WS * VS
    ident[ii, 1, ii] = WS * VS

    common = dict(
        wqk8=np.ascontiguousarray(
            wqk_t.reshape(DCH, P, 2 * DCH, P).transpose(2, 1, 0, 3)).astype(NFP8),
        wv8=np.ascontiguousarray(
            wv_t.reshape(DCH, P, 2, 512).transpose(2, 1, 0, 3)).astype(NFP8),
        wo8=np.ascontiguousarray(
            (wo_t * WS).reshape(DCH, P, D).transpose(1, 0, 2)).astype(NFP8),
        ident8=ident,
        bqk=np.ascontiguousarray(
            bqk_f.reshape(2 * DCH, P).T).astype(np.float32),
        bo2=np.stack([bo2, np.zeros_like(bo2)])[None].astype(NFP8),
        lnw=ln_w, lnb=ln_b)

    # token order per cluster
    sort_tok = np.argsort(cid, kind="stable")
    cl_start = np.concatenate([[0], np.cumsum(counts)]).astype(int)

    in_maps = []
    slot_tok = []
    for core in range(NCORES):
        xp = np.zeros((T, D), dtype=np.float32)
        pm = np.ones((NB, P), dtype=np.float32)
        slots = []
        toks = []
        for j in range(NB):
            cl = int(assign[core, j])
            nk = int(counts[cl])
            tk = sort_tok[cl_start[cl]:cl_start[cl] + nk]
            sl = np.arange(int(off[j]), int(off[j]) + nk)
            xp[sl] = x[tk]
            pm[j, :nk] = 0.0
            slots.append(sl)
            toks.append(tk)
        im = dict(common)
        xT = np.ascontiguousarray(xp.T)  # [D, T]
        hi, lo = _hi_lo(xT)
        x8 = np.empty((P, DCH, 2, T), dtype=NFP8)
        x8[:, :, 0, :] = hi.reshape(DCH, P, T).transpose(1, 0, 2)
        x8[:, :, 1, :] = lo.reshape(DCH, P, T).transpose(1, 0, 2)
        im["xT8"] = x8
        im["pmask"] = pm[None].astype(NBF16)
        in_maps.append(im)
        slot_tok.append((np.concatenate(slots), np.concatenate(toks)))

    res = run_bass_kernel_spmd(nc, in_maps, core_ids=list(range(NCORES)),
                               trace=TRACE)
    kernel.last_results = res

    out = np.empty((N, D), dtype=np.float32)
    for core in range(NCORES):
        slots, toks = slot_tok[core]
        out[toks] = res.results[core]["y"][slots]
    return out


# revision 64
# speedup vs baseline: 1.0194x; 1.0026x over previous
"""MaskClusterAttention Trainium2 kernel (fp8 DoubleRow redesign).

Sparse attention: tokens attend only within their cluster (64 clusters,
~64 tokens each).  Host sorts clusters by size and assigns rank
8j+c -> (core c, block j), so all 8 cores share one SPMD program with
per-block compile-time widths S_j = max size in rank-group j (>=64,
mult of 4).  Per-core padding is ~9% instead of the 50% a fixed 96-pad
costs.

Per core (T = sum S_j tokens):
  S1: q,k projections emitted transposed [128, T] via fp8e4 DoubleRow
      matmuls (2x128 contraction per instruction, 0.5 cyc/row); v per
      block in [S_j, 1024] layout.  x is staged as interleaved fp8
      hi/lo pairs [128, 8, 2, T] so single-fp8 (hi) and compensated
      (hi+lo) passes share one layout.  q scaled by 1/sqrt(hd) and in_proj
      bias applied during the PSUM->SBUF copy (Act, per-partition bias).
  S2: per (head-pair, block): scoresT [S_j, 2, S_j] in PSUM; pad-key
      masking is a rank-1 matmul (padflag x -1e4 row) accumulated into
      the same PSUM group, so exp needs no bias and merges 4 blocks per
      Act op.  Row sums via gpsimd partition_all_reduce, reciprocal
      (DVE, bf16), normalize-multiply fused with the fp8 downcast of
      ctxT.
  S3: out_proj via fp8 DoubleRow from ctxT8; residual x and bias bo'
      (bo + bv@Wo, v-bias folded out) enter the same PSUM via identity /
      ones rank-1 matmuls.  LayerNorm stats come free from Act accum_out
      (sum y, sum y^2), apply via one tensor_scalar.  No collectives.

Host scatters per-core outputs back through the slot map.
"""

from contextlib import ExitStack

import ml_dtypes
import numpy as np

import concourse.bass as bass
import concourse.bass_isa as bass_isa
import concourse.mybir as mybir
import concourse.tile as tile
from concourse import bacc
from concourse.bass_utils import run_bass_kernel_spmd

F32 = mybir.dt.float32
BF16 = mybir.dt.bfloat16
FP8 = mybir.dt.float8e4
NFP8 = ml_dtypes.float8_e4m3
NBF16 = ml_dtypes.bfloat16

N, D, H, HD, NCLUST, NCORES = 4096, 1024, 16, 64, 64, 8
NB = NCLUST // NCORES  # blocks (clusters) per core
P = 128
DCH = D // P  # 8 contraction chunks
NEG = -10000.0
WS = 64.0  # fp8 weight upscale (keeps quanta out of e4m3 subnormals)
VS = 2.0   # v / ctx storage scale

TRACE = False
_cache = {}


def build_program(S, apply_ln_affine, qk_lo=False, v_lo=False):
    """S: tuple of NB block widths (each >=64, mult of 4)."""
    S = list(S)
    off = np.concatenate([[0], np.cumsum(S)]).astype(int)  # block offsets
    T = int(off[-1])
    MT = (T + P - 1) // P  # 128-token tiles for S3
    QUADS = [(0, 4), (4, 8)]  # block groups for S2 merging
    CWMAX = max(sum(S[b0:b1]) for b0, b1 in QUADS)

    def pack_banks(sizes, bank=512, cap=1024):
        """Greedy offsets so no region crosses a 512-float psum bank."""
        offs, o = [], 0
        for s in sizes:
            if o // bank != (o + s - 1) // bank:
                o = (o // bank + 1) * bank
            offs.append(o)
            o += s
        assert o <= cap, f"psum pack overflow: {sizes}"
        return offs, o

    FWMAX = max(pack_banks([S[b] for b in range(b0, b1)
                            for _ in range(2)])[1] for b0, b1 in QUADS)
    TH0 = min(512, T)

    nc = bacc.Bacc("TRN2", target_bir_lowering=False, debug=False,
                   num_devices=NCORES)

    xT8 = nc.dram_tensor("xT8", [P, DCH, 2, T], FP8, kind="ExternalInput").ap()
    wqk8 = nc.dram_tensor("wqk8", [2 * DCH, P, DCH * P], FP8,
                          kind="ExternalInput").ap()
    wv8 = nc.dram_tensor("wv8", [2, P, DCH, 512], FP8,
                         kind="ExternalInput").ap()
    wo8 = nc.dram_tensor("wo8", [P, DCH, D], FP8, kind="ExternalInput").ap()
    ident8 = nc.dram_tensor("ident8", [P, 2, P], FP8, kind="ExternalInput").ap()
    bqk = nc.dram_tensor("bqk", [P, 2 * DCH], F32, kind="ExternalInput").ap()
    bo2 = nc.dram_tensor("bo2", [1, 2, D], FP8, kind="ExternalInput").ap()
    pmask = nc.dram_tensor("pmask", [1, NB, P], BF16, kind="ExternalInput").ap()
    lnw = nc.dram_tensor("lnw", [D], F32, kind="ExternalInput").ap()
    lnb = nc.dram_tensor("lnb", [D], F32, kind="ExternalInput").ap()
    y = nc.dram_tensor("y", [T, D], F32, kind="ExternalOutput").ap()

    DR = mybir.MatmulPerfMode.DoubleRow
    EXP = mybir.ActivationFunctionType.Exp
    CPY = mybir.ActivationFunctionType.Copy
    IDN = mybir.ActivationFunctionType.Identity
    SQR = mybir.ActivationFunctionType.Square
    SQRT = mybir.ActivationFunctionType.Sqrt

    with tile.TileContext(nc) as tc, ExitStack() as es:
        es.enter_context(nc.allow_low_precision(
            reason="fp8 ctx / bf16 scratch are intentional"))
        singles = es.enter_context(tc.tile_pool(name="singles", bufs=1))
        qkpool = es.enter_context(tc.tile_pool(name="qkpool", bufs=16))
        vpool = es.enter_context(tc.tile_pool(name="vpool", bufs=NB))
        ctxpool = es.enter_context(tc.tile_pool(name="ctxpool", bufs=1))
        exmpool = es.enter_context(tc.tile_pool(name="exmpool", bufs=3))
        sumpool = es.enter_context(tc.tile_pool(name="sumpool", bufs=2))
        rbpool = es.enter_context(tc.tile_pool(name="rbpool", bufs=3))
        s3pool = es.enter_context(tc.tile_pool(name="s3pool", bufs=4))
        # y_t tiles are per-m tags (MT live at once); yo rotates via bufs
        smallp = es.enter_context(tc.tile_pool(name="smallp", bufs=6))
        wpool = es.enter_context(tc.tile_pool(name="wpool", bufs=3))
        wvpool = es.enter_context(tc.tile_pool(name="wvpool", bufs=1))
        ps2 = es.enter_context(tc.tile_pool(name="ps2", bufs=3, space="PSUM"))
        ps1 = es.enter_context(tc.tile_pool(name="ps1", bufs=2, space="PSUM"))

        # --- resident inputs / constants (DMA order = need order) ---
        xT_sb = singles.tile([P, DCH, 2, T], FP8, tag="xT")
        nc.sync.dma_start(out=xT_sb[:, :, 0, :],
                          in_=xT8[:, :, 0, :])
        wo_sb = singles.tile([P, DCH, D], FP8, tag="wo")
        id_sb = singles.tile([P, 2, P], FP8, tag="ident")
        bqk_sb = singles.tile([P, 2 * DCH], F32, tag="bqk")
        nc.sync.dma_start(out=bqk_sb, in_=bqk)
        bo_sb = singles.tile([1, 2, D], FP8, tag="bo2")
        nc.sync.dma_start(out=bo_sb, in_=bo2)
        pm_sb = singles.tile([1, NB, P], BF16, tag="pmask")
        nc.sync.dma_start(out=pm_sb, in_=pmask)
        neg_sb = singles.tile([1, P], BF16, tag="negrow")
        nc.vector.memset(neg_sb, NEG)
        ones_sb = singles.tile([1, 2, P], FP8, tag="onesrow")
        nc.vector.memset(ones_sb[:, 0, :], 1.0)
        nc.vector.memset(ones_sb[:, 1, :], 0.0)
        eps_sb = singles.tile([P, 1], F32, tag="eps")
        nc.vector.memset(eps_sb, 1e-5)
        if apply_ln_affine:
            lnw_sb = singles.tile([P, D], F32, tag="lnw")
            nc.gpsimd.dma_start(out=lnw_sb, in_=bass.AP(
                tensor=lnw.tensor, offset=lnw.offset, ap=[[0, P], *lnw.ap]))
            lnb_sb = singles.tile([P, D], F32, tag="lnb")
            nc.gpsimd.dma_start(out=lnb_sb, in_=bass.AP(
                tensor=lnb.tensor, offset=lnb.offset, ap=[[0, P], *lnb.ap]))

        qkT_sb = [qkpool.tile([P, T], BF16, tag="qkT", name=f"qkT{i}")
                  for i in range(2 * DCH)]
        v_sb = [vpool.tile([S[b], D], BF16, tag=f"v{b}", name=f"v{b}")
                for b in range(NB)]
        ctx_sb = ctxpool.tile([P, DCH, T], FP8, tag="ctx8")

        THS = [(0, TH0)] + ([(TH0, T)] if T > TH0 else [])

        def emit_qk(wi):
            """q or k chunk wi (0-7 q, 8-15 k) -> qkT_sb[wi] bf16 [128, T]."""
            wt = wpool.tile([P, DCH, P], FP8, tag="wt", name=f"wt{wi}")
            nc.sync.dma_start(out=wt.rearrange("p c w -> p (c w)"),
                              in_=wqk8[wi])
            ps = ps2.tile([P, 1024], F32, tag="ps", name=f"qk{wi}")
            lo, hi = (0, 2) if qk_lo else (0, 1)
            for t0, t1 in THS:
                first = True
                for c in range(0, DCH, 2):
                    for li in range(lo, hi):
                        nc.tensor.matmul(
                            ps[:, t0:t1],
                            wt[:, c:c + 2, :],
                            xT_sb[:, c:c + 2, li, t0:t1],
                            start=first, stop=(c == DCH - 2 and li == hi - 1),
                            perf_mode=DR)
                        first = False
            qsc = (1.0 / np.sqrt(HD) if wi < DCH else 1.0) / WS
            if wi in (0, 8) and T > TH0:
                nc.scalar.activation(out=qkT_sb[wi][:, 0:TH0],
                                     in_=ps[:, 0:TH0], func=IDN,
                                     bias=bqk_sb[:, wi:wi + 1], scale=qsc)
                nc.scalar.activation(out=qkT_sb[wi][:, TH0:T],
                                     in_=ps[:, TH0:T], func=IDN,
                                     bias=bqk_sb[:, wi:wi + 1], scale=qsc)
            else:
                nc.scalar.activation(out=qkT_sb[wi], in_=ps[:, 0:T], func=IDN,
                                     bias=bqk_sb[:, wi:wi + 1], scale=qsc)

        def emit_v(b, oh, wvt, eng):
            """v half oh for block b -> v_sb[b][:, oh*512:] (bias folded)."""
            bs = slice(int(off[b]), int(off[b + 1]))
            oc = slice(oh * 512, (oh + 1) * 512)
            ps = ps1.tile([P, 512], F32, tag="cx", name=f"v{b}_{oh}")
            lo, hi = (0, 2) if v_lo else (0, 1)
            first = True
            for c in range(0, DCH, 2):
                for li in range(lo, hi):
                    nc.tensor.matmul(
                        ps[0:S[b], :],
                        xT_sb[:, c:c + 2, li, bs],
                        wvt[:, c:c + 2, :],
                        start=first, stop=(c == DCH - 2 and li == hi - 1),
                        perf_mode=DR)
                    first = False
            if eng == "act":
                nc.scalar.activation(out=v_sb[b][:, oc], in_=ps[0:S[b], :],
                                     func=CPY, scale=VS / WS)
            else:
                nc.vector.tensor_scalar(
                    out=v_sb[b][:, oc], in0=ps[0:S[b], :], scalar1=VS / WS,
                    scalar2=None, op0=mybir.AluOpType.mult)

        def emit_s2(hp, quad):
            """head-pair hp, blocks quad[0]:quad[1] (4 blocks)."""
            b0, b1 = quad
            qa, ka = qkT_sb[hp], qkT_sb[DCH + hp]
            # per-block score regions of width 2*S_b, packed to psum banks
            roffs, FW = pack_banks([2 * S[b] for b in range(b0, b1)])
            qoff = {(b, j): roffs[b - b0] + j * S[b]
                    for b in range(b0, b1) for j in range(2)}
            kpmax = max(S[b0:b1])
            sc = ps2.tile([P, 1024], F32, tag="ps", name=f"sc{hp}_{b0}")
            for b in range(b0, b1):
                bs = slice(int(off[b]), int(off[b + 1]))
                for j in range(2):
                    rows = slice(j * HD, (j + 1) * HD)
                    reg = slice(qoff[b, j], qoff[b, j] + S[b])
                    nc.tensor.matmul(sc[0:S[b], reg], ka[rows, bs],
                                     qa[rows, bs], start=True, stop=False)
                    nc.tensor.matmul(sc[0:S[b], reg],
                                     pm_sb[:, b, 0:S[b]], neg_sb[:, 0:S[b]],
                                     start=False, stop=True)
            exm = exmpool.tile([P, FWMAX], BF16, tag="exm")
            nc.scalar.activation(out=exm[0:kpmax, 0:FW], in_=sc[0:kpmax, 0:FW],
                                 func=EXP)
            # row sums over keys (partitions) -> all partitions hold sum
            sums = sumpool.tile([P, 2, CWMAX], BF16, tag="sums")
            coff = [0] * NB  # free offsets of S_b regions in cx psum
            o = 0
            for b in range(b0, b1):
                coff[b] = o
                o += S[b]
            CW = o
            for b in range(b0, b1):
                nc.gpsimd.partition_all_reduce(
                    sums[0:S[b], :, coff[b]:coff[b] + S[b]],
                    exm[0:S[b], qoff[b, 0]:qoff[b, 0] + 2 * S[b]].rearrange(
                        "k (j q) -> k j q", j=2),
                    channels=S[b], reduce_op=bass_isa.ReduceOp.add)
            rb = rbpool.tile([P, CWMAX], BF16, tag="rb")
            nc.vector.reciprocal(out=rb[0:HD, 0:CW], in_=sums[0:HD, 0, 0:CW])
            nc.vector.reciprocal(out=rb[HD:P, 0:CW], in_=sums[0:HD, 1, 0:CW])
            cx = ps1.tile([P, 512], F32, tag="cx", name=f"cx{hp}_{b0}")
            for b in range(b0, b1):
                for j in range(2):
                    h = 2 * hp + j
                    reg = slice(qoff[b, j], qoff[b, j] + S[b])
                    nc.tensor.matmul(
                        cx[j * HD:(j + 1) * HD, coff[b]:coff[b] + S[b]],
                        v_sb[b][:, h * HD:(h + 1) * HD], exm[0:S[b], reg],
                        start=True, stop=True,
                        tile_position=(0, j * HD))
            ts = slice(int(off[b0]), int(off[b1]))
            nc.vector.tensor_mul(out=ctx_sb[:, hp, ts], in0=cx[:, 0:CW],
                                 in1=rb[:, 0:CW])

        y_ts = [singles.tile([P, D], BF16, tag=f"y{m}", name=f"y_t{m}")
                for m in range(MT)]
        sys_ = [singles.tile([P, 4], F32, tag=f"sy{m}", name=f"sy{m}")
                for m in range(MT)]

        def emit_s3a(m):
            """out_proj + residual + bias + stat sums for token tile m."""
            pt = min(P, T - m * P)
            mc = slice(m * P, m * P + pt)
            ps = ps2.tile([P, 1024], F32, tag="ps", name=f"s3_{m}")
            for oh in range(2):
                oc = slice(oh * 512, (oh + 1) * 512)
                for c in range(0, DCH, 2):
                    nc.tensor.matmul(ps[0:pt, oc],
                                     ctx_sb[:, c:c + 2, mc],
                                     wo_sb[:, c:c + 2, oc],
                                     start=(c == 0), stop=False,
                                     perf_mode=DR)
                # residual: += x (hi+lo) for the 4 ident chunks of this half
                for c in range(oh * 4, oh * 4 + 4):
                    nc.tensor.matmul(ps[0:pt, c * P:(c + 1) * P],
                                     xT_sb[:, c, :, mc], id_sb,
                                     start=False, stop=False, perf_mode=DR)
                # bias row (DR: row0 = ones x bo2, row1 = zeros)
                nc.tensor.matmul(ps[0:pt, oc], ones_sb[:, :, 0:pt],
                                 bo_sb[:, :, oc], start=False, stop=True,
                                 perf_mode=DR)
            y_t, sy = y_ts[m], sys_[m]
            nc.scalar.activation(out=y_t[0:pt], in_=ps[0:pt], func=CPY,
                                 scale=1.0 / (WS * VS),
                                 accum_out=sy[0:pt, 0:1])
            scr = singles.tile([P, D], BF16, tag="scr")
            nc.vector.scalar_tensor_tensor(
                out=scr[0:pt], in0=y_t[0:pt], scalar=0.0, in1=y_t[0:pt],
                op0=mybir.AluOpType.add, op1=mybir.AluOpType.mult,
                accum_out=sy[0:pt, 1:2])

        def emit_s3b(m):
            """LayerNorm apply + store for token tile m."""
            pt = min(P, T - m * P)
            mc = slice(m * P, m * P + pt)
            y_t, sy = y_ts[m], sys_[m]
            # mu = sy/D ; var = sq/D - mu^2 ; rstd = 1/sqrt(var+eps)
            mv = smallp.tile([P, 4], F32, tag="mv")
            nc.scalar.activation(out=mv[0:pt, 0:2], in_=sy[0:pt, 0:2],
                                 func=CPY, scale=1.0 / D)
            nc.vector.tensor_tensor(out=mv[0:pt, 2:3], in0=mv[0:pt, 0:1],
                                    in1=mv[0:pt, 0:1], op=mybir.AluOpType.mult)
            nc.vector.tensor_tensor(out=mv[0:pt, 3:4], in0=mv[0:pt, 1:2],
                                    in1=mv[0:pt, 2:3],
                                    op=mybir.AluOpType.subtract)
            rstd = smallp.tile([P, 2], F32, tag="rstd")
            nc.scalar.activation(out=rstd[0:pt, 1:2], in_=mv[0:pt, 3:4],
                                 func=SQRT, bias=eps_sb[0:pt], scale=1.0)
            nc.vector.reciprocal(out=rstd[0:pt, 1:2], in_=rstd[0:pt, 1:2])
            for h in range(2):
                hc = slice(h * 512, (h + 1) * 512)
                yo = s3pool.tile([P, 512], F32, tag="yo")
                nc.vector.tensor_scalar(
                    out=yo[0:pt], in0=y_t[0:pt, hc], scalar1=mv[0:pt, 0:1],
                    scalar2=rstd[0:pt, 1:2], op0=mybir.AluOpType.subtract,
                    op1=mybir.AluOpType.mult)
                if apply_ln_affine:
                    nc.vector.tensor_mul(out=yo[0:pt], in0=yo[0:pt],
                                         in1=lnw_sb[0:pt, hc])
                    nc.vector.tensor_add(out=yo[0:pt], in0=yo[0:pt],
                                         in1=lnb_sb[0:pt, hc])
                nc.sync.dma_start(out=y[mc, hc], in_=yo[0:pt])

        # --- emission schedule: interleave S1 qk/v with S2 so the vector
        # engines start early; S3 after all S2 (ctx complete). ---
        for wi in (0, 8, 1, 9):
            emit_qk(wi)
        wv0 = wvpool.tile([P, DCH, 512], FP8, tag="wv", name="wv0")
        nc.sync.dma_start(out=wv0, in_=wv8[0])
        for b in range(NB):
            emit_v(b, 0, wv0, "act" if b % 2 else "dve")
        emit_s2(0, QUADS[0])
        emit_qk(2)
        emit_qk(10)
        emit_s2(1, QUADS[0])
        emit_qk(3)
        emit_qk(11)
        emit_s2(2, QUADS[0])
        emit_qk(4)
        emit_qk(12)
        emit_s2(3, QUADS[0])
        wv1 = wvpool.tile([P, DCH, 512], FP8, tag="wv", name="wv1")
        nc.sync.dma_start(out=wv1, in_=wv8[1])
        nc.sync.dma_start(out=xT_sb[:, :, 1, :], in_=xT8[:, :, 1, :])
        nc.sync.dma_start(out=wo_sb, in_=wo8)
        nc.sync.dma_start(out=id_sb, in_=ident8)
        for b in range(NB):
            emit_v(b, 1, wv1, "act" if b % 2 else "dve")
        emit_qk(5)
        emit_qk(13)
        emit_s2(4, QUADS[0])
        emit_qk(6)
        emit_qk(14)
        emit_s2(5, QUADS[0])
        emit_qk(7)
        emit_qk(15)
        emit_s2(6, QUADS[0])
        emit_s2(7, QUADS[0])
        emit_s2(0, QUADS[1])
        emit_s3a(0)
        emit_s2(1, QUADS[1])
        emit_s3a(1)
        for hp in range(2, 8):
            emit_s2(hp, QUADS[1])
        emit_s3b(0)
        emit_s3b(1)
        for m in range(2, MT):
            emit_s3a(m)
        for m in range(2, MT):
            emit_s3b(m)

    nc.compile()
    return nc


def _numpy_fallback(x, cluster_id, in_proj_w, in_proj_b, out_proj_w,
                    out_proj_b, ln_w, ln_b, num_heads):
    n, d = x.shape
    hd = d // num_heads
    x64 = x.astype(np.float64)
    qkv = x64 @ in_proj_w.T.astype(np.float64) + in_proj_b
    q, k, v = np.split(qkv, 3, axis=-1)
    q = q.reshape(n, num_heads, hd)
    k = k.reshape(n, num_heads, hd)
    v = v.reshape(n, num_heads, hd)
    valid = cluster_id >= 0
    allow = (cluster_id[:, None] == cluster_id[None, :]) & valid[:, None] & valid[None, :]
    scores = np.einsum("nhd,mhd->hnm", q, k) / np.sqrt(hd)
    scores = np.where(allow[None], scores, np.finfo(np.float32).min)
    scores -= scores.max(axis=-1, keepdims=True)
    e = np.exp(scores)
    attn = e / e.sum(axis=-1, keepdims=True)
    ctx = np.einsum("hnm,mhd->nhd", attn, v).reshape(n, d)
    yv = x64 + ctx @ out_proj_w.T.astype(np.float64) + out_proj_b
    mu = yv.mean(-1, keepdims=True)
    var = ((yv - mu) ** 2).mean(-1, keepdims=True)
    out = (yv - mu) / np.sqrt(var + 1e-5) * ln_w + ln_b
    return out.astype(np.float32)


def _hi_lo(a):
    hi = a.astype(NFP8)
    lo = (a - hi.astype(np.float32)).astype(NFP8)
    return hi, lo


def kernel(x, cluster_id, in_proj_w, in_proj_b, out_proj_w, out_proj_b,
           ln_w, ln_b, num_heads):
    x = np.asarray(x, dtype=np.float32)
    cid = np.asarray(cluster_id).astype(np.int64)
    in_proj_w = np.asarray(in_proj_w, dtype=np.float32)
    in_proj_b = np.asarray(in_proj_b, dtype=np.float32)
    out_proj_w = np.asarray(out_proj_w, dtype=np.float32)
    out_proj_b = np.asarray(out_proj_b, dtype=np.float32)
    ln_w = np.asarray(ln_w, dtype=np.float32)
    ln_b = np.asarray(ln_b, dtype=np.float32)
    nh = int(np.asarray(num_heads))

    counts = np.bincount(cid, minlength=NCLUST) if cid.size and cid.min() >= 0 else None
    if (x.shape != (N, D) or nh != H or counts is None
            or cid.max() >= NCLUST or counts.max() > 120):
        return _numpy_fallback(x, cid, in_proj_w, in_proj_b, out_proj_w,
                               out_proj_b, ln_w, ln_b, nh)

    # --- cluster -> (core, block) assignment: sort clusters by size desc,
    # rank 8j+c -> core c block j; S_j = max(ceil4(max size in group), 64).
    order_cl = np.argsort(-counts, kind="stable")
    assign = np.empty((NCORES, NB), dtype=np.int64)
    S = []
    for j in range(NB):
        grp = order_cl[j * NCORES:(j + 1) * NCORES]
        assign[:, j] = grp
        S.append(max(int(-(-int(counts[grp].max()) // 4) * 4), 64))
    S = tuple(S)
    off = np.concatenate([[0], np.cumsum(S)]).astype(int)
    T = int(off[-1])

    ln_trivial = bool(np.all(ln_w == 1.0) and np.all(ln_b == 0.0))
    key = (S, not ln_trivial)
    if key not in _cache:
        _cache[key] = build_program(S, not ln_trivial)
    nc = _cache[key]

    # --- shared (replicated) weight prep ---
    scale = 1.0 / np.sqrt(HD)
    wqk_t = np.ascontiguousarray(in_proj_w[:2 * D].T) * WS  # [D, 2D]
    bqk_f = in_proj_b[:2 * D].copy()
    bqk_f[:D] *= scale
    wv_t = np.ascontiguousarray(in_proj_w[2 * D:].T) * WS  # [D, D]
    bv = in_proj_b[2 * D:]
    wo_t = np.ascontiguousarray(out_proj_w.T)  # [D, D]
    bo2 = ((out_proj_b + bv @ wo_t) * (WS * VS)).astype(np.float32)

    ident = np.zeros((P, 2, P), dtype=NFP8)
    ii = np.arange(P)
    ident[ii, 0, ii] = WS * VS
    ident[ii, 1, ii] = WS * VS

    common = dict(
        wqk8=np.ascontiguousarray(
            wqk_t.reshape(DCH, P, 2 * DCH, P).transpose(2, 1, 0, 3)).astype(NFP8),
        wv8=np.ascontiguousarray(
            wv_t.reshape(DCH, P, 2, 512).transpose(2, 1, 0, 3)).astype(NFP8),
        wo8=np.ascontiguousarray(
            (wo_t * WS).reshape(DCH, P, D).transpose(1, 0, 2)).astype(NFP8),
        ident8=ident,
        bqk=np.ascontiguousarray(
            bqk_f.reshape(2 * DCH, P).T).astype(np.float32),
        bo2=np.stack([bo2, np.zeros_like(bo2)])[None].astype(NFP8),
        lnw=ln_w, lnb=ln_b)

    # token order per cluster
    sort_tok = np.argsort(cid, kind="stable")
    cl_start = np.concatenate([[0], np.cumsum(counts)]).astype(int)

    in_maps = []
    slot_tok = []
    for core in range(NCORES):
        xp = np.zeros((T, D), dtype=np.float32)
        pm = np.ones((NB, P), dtype=np.float32)
        slots = []
        toks = []
        for j in range(NB):
            cl = int(assign[core, j])
            nk = int(counts[cl])
            tk = sort_tok[cl_start[cl]:cl_start[cl] + nk]
            sl = np.arange(int(off[j]), int(off[j]) + nk)
            xp[sl] = x[tk]
            pm[j, :nk] = 0.0
            slots.append(sl)
            toks.append(tk)
        im = dict(common)
        xT = np.ascontiguousarray(xp.T)  # [D, T]
        hi, lo = _hi_lo(xT)
        x8 = np.empty((P, DCH, 2, T), dtype=NFP8)
        x8[:, :, 0, :] = hi.reshape(DCH, P, T).transpose(1, 0, 2)
        x8[:, :, 1, :] = lo.reshape(DCH, P, T).transpose(1, 0, 2)
        im["xT8"] = x8
        im["pmask"] = pm[None].astype(NBF16)
        in_maps.append(im)
        slot_tok.append((np.concatenate(slots), np.concatenate(toks)))

    res = run_bass_kernel_spmd(nc, in_maps, core_ids=list(range(NCORES)),
                               trace=TRACE)
    kernel.last_results = res

    out = np.empty((N, D), dtype=np.float32)
    for core in range(NCORES):
        slots, toks = slot_tok[core]
        out[toks] = res.results[core]["y"][slots]
    return out


# revision 65
# speedup vs baseline: 1.0277x; 1.0082x over previous
"""MaskClusterAttention Trainium2 kernel (fp8 DoubleRow redesign).

Sparse attention: tokens attend only within their cluster (64 clusters,
~64 tokens each).  Host sorts clusters by size and assigns rank
8j+c -> (core c, block j), so all 8 cores share one SPMD program with
per-block compile-time widths S_j = max size in rank-group j (>=64,
mult of 4).  Per-core padding is ~9% instead of the 50% a fixed 96-pad
costs.

Per core (T = sum S_j tokens):
  S1: q,k projections emitted transposed [128, T] via fp8e4 DoubleRow
      matmuls (2x128 contraction per instruction, 0.5 cyc/row); v per
      block in [S_j, 1024] layout.  x is staged as interleaved fp8
      hi/lo pairs [128, 8, 2, T] so single-fp8 (hi) and compensated
      (hi+lo) passes share one layout.  q scaled by 1/sqrt(hd) and in_proj
      bias applied during the PSUM->SBUF copy (Act, per-partition bias).
  S2: per (head-pair, block): scoresT [S_j, 2, S_j] in PSUM; pad-key
      masking is a rank-1 matmul (padflag x -1e4 row) accumulated into
      the same PSUM group, so exp needs no bias and merges 4 blocks per
      Act op.  Row sums via gpsimd partition_all_reduce, reciprocal
      (DVE, bf16), normalize-multiply fused with the fp8 downcast of
      ctxT.
  S3: out_proj via fp8 DoubleRow from ctxT8; residual x and bias bo'
      (bo + bv@Wo, v-bias folded out) enter the same PSUM via identity /
      ones rank-1 matmuls.  LayerNorm stats come free from Act accum_out
      (sum y, sum y^2), apply via one tensor_scalar.  No collectives.

Host scatters per-core outputs back through the slot map.
"""

from contextlib import ExitStack

import ml_dtypes
import numpy as np

import concourse.bass as bass
import concourse.bass_isa as bass_isa
import concourse.mybir as mybir
import concourse.tile as tile
from concourse import bacc
from concourse.bass_utils import run_bass_kernel_spmd

F32 = mybir.dt.float32
BF16 = mybir.dt.bfloat16
FP8 = mybir.dt.float8e4
NFP8 = ml_dtypes.float8_e4m3
NBF16 = ml_dtypes.bfloat16

N, D, H, HD, NCLUST, NCORES = 4096, 1024, 16, 64, 64, 8
NB = NCLUST // NCORES  # blocks (clusters) per core
P = 128
DCH = D // P  # 8 contraction chunks
NEG = -10000.0
WS = 64.0  # fp8 weight upscale (keeps quanta out of e4m3 subnormals)
VS = 2.0   # v / ctx storage scale

TRACE = False
_cache = {}


def build_program(S, apply_ln_affine, qk_lo=False, v_lo=False):
    """S: tuple of NB block widths (each >=64, mult of 4)."""
    S = list(S)
    off = np.concatenate([[0], np.cumsum(S)]).astype(int)  # block offsets
    T = int(off[-1])
    MT = (T + P - 1) // P  # 128-token tiles for S3
    QUADS = [(0, 4), (4, 8)]  # block groups for S2 merging
    CWMAX = max(sum(S[b0:b1]) for b0, b1 in QUADS)

    def pack_banks(sizes, bank=512, cap=1024):
        """Greedy offsets so no region crosses a 512-float psum bank."""
        offs, o = [], 0
        for s in sizes:
            if o // bank != (o + s - 1) // bank:
                o = (o // bank + 1) * bank
            offs.append(o)
            o += s
        assert o <= cap, f"psum pack overflow: {sizes}"
        return offs, o

    FWMAX = max(pack_banks([S[b] for b in range(b0, b1)
                            for _ in range(2)])[1] for b0, b1 in QUADS)
    TH0 = min(512, T)

    nc = bacc.Bacc("TRN2", target_bir_lowering=False, debug=False,
                   num_devices=NCORES)

    xT8 = nc.dram_tensor("xT8", [P, DCH, 2, T], FP8, kind="ExternalInput").ap()
    wqk8 = nc.dram_tensor("wqk8", [2 * DCH, P, DCH * P], FP8,
                          kind="ExternalInput").ap()
    wv8 = nc.dram_tensor("wv8", [2, P, DCH, 512], FP8,
                         kind="ExternalInput").ap()
    wo8 = nc.dram_tensor("wo8", [P, DCH, D], FP8, kind="ExternalInput").ap()
    ident8 = nc.dram_tensor("ident8", [P, 2, P], FP8, kind="ExternalInput").ap()
    bqk = nc.dram_tensor("bqk", [P, 2 * DCH], F32, kind="ExternalInput").ap()
    bo2 = nc.dram_tensor("bo2", [1, 2, D], FP8, kind="ExternalInput").ap()
    pmask = nc.dram_tensor("pmask", [1, NB, P], BF16, kind="ExternalInput").ap()
    lnw = nc.dram_tensor("lnw", [D], F32, kind="ExternalInput").ap()
    lnb = nc.dram_tensor("lnb", [D], F32, kind="ExternalInput").ap()
    y = nc.dram_tensor("y", [T, D], F32, kind="ExternalOutput").ap()

    DR = mybir.MatmulPerfMode.DoubleRow
    EXP = mybir.ActivationFunctionType.Exp
    CPY = mybir.ActivationFunctionType.Copy
    IDN = mybir.ActivationFunctionType.Identity
    SQR = mybir.ActivationFunctionType.Square
    SQRT = mybir.ActivationFunctionType.Sqrt

    with tile.TileContext(nc) as tc, ExitStack() as es:
        es.enter_context(nc.allow_low_precision(
            reason="fp8 ctx / bf16 scratch are intentional"))
        singles = es.enter_context(tc.tile_pool(name="singles", bufs=1))
        qkpool = es.enter_context(tc.tile_pool(name="qkpool", bufs=16))
        vpool = es.enter_context(tc.tile_pool(name="vpool", bufs=NB))
        ctxpool = es.enter_context(tc.tile_pool(name="ctxpool", bufs=1))
        exmpool = es.enter_context(tc.tile_pool(name="exmpool", bufs=3))
        sumpool = es.enter_context(tc.tile_pool(name="sumpool", bufs=2))
        rbpool = es.enter_context(tc.tile_pool(name="rbpool", bufs=3))
        s3pool = es.enter_context(tc.tile_pool(name="s3pool", bufs=4))
        # y_t tiles are per-m tags (MT live at once); yo rotates via bufs
        smallp = es.enter_context(tc.tile_pool(name="smallp", bufs=6))
        wpool = es.enter_context(tc.tile_pool(name="wpool", bufs=3))
        wvpool = es.enter_context(tc.tile_pool(name="wvpool", bufs=1))
        ps2 = es.enter_context(tc.tile_pool(name="ps2", bufs=3, space="PSUM"))
        ps1 = es.enter_context(tc.tile_pool(name="ps1", bufs=2, space="PSUM"))

        # --- resident inputs / constants (DMA order = need order) ---
        xT_sb = singles.tile([P, DCH, 2, T], FP8, tag="xT")
        nc.sync.dma_start(out=xT_sb[:, :, 0, :],
                          in_=xT8[:, :, 0, :])
        wo_sb = singles.tile([P, DCH, D], FP8, tag="wo")
        id_sb = singles.tile([P, 2, P], FP8, tag="ident")
        bqk_sb = singles.tile([P, 2 * DCH], F32, tag="bqk")
        nc.sync.dma_start(out=bqk_sb, in_=bqk)
        bo_sb = singles.tile([1, 2, D], FP8, tag="bo2")
        nc.sync.dma_start(out=bo_sb, in_=bo2)
        pm_sb = singles.tile([1, NB, P], BF16, tag="pmask")
        nc.sync.dma_start(out=pm_sb, in_=pmask)
        neg_sb = singles.tile([1, P], BF16, tag="negrow")
        nc.vector.memset(neg_sb, NEG)
        ones_sb = singles.tile([1, 2, P], FP8, tag="onesrow")
        nc.vector.memset(ones_sb[:, 0, :], 1.0)
        nc.vector.memset(ones_sb[:, 1, :], 0.0)
        eps_sb = singles.tile([P, 1], F32, tag="eps")
        nc.vector.memset(eps_sb, 1e-5)
        if apply_ln_affine:
            lnw_sb = singles.tile([P, D], F32, tag="lnw")
            nc.gpsimd.dma_start(out=lnw_sb, in_=bass.AP(
                tensor=lnw.tensor, offset=lnw.offset, ap=[[0, P], *lnw.ap]))
            lnb_sb = singles.tile([P, D], F32, tag="lnb")
            nc.gpsimd.dma_start(out=lnb_sb, in_=bass.AP(
                tensor=lnb.tensor, offset=lnb.offset, ap=[[0, P], *lnb.ap]))

        qkT_sb = [qkpool.tile([P, T], BF16, tag="qkT", name=f"qkT{i}")
                  for i in range(2 * DCH)]
        v_sb = [vpool.tile([S[b], D], BF16, tag=f"v{b}", name=f"v{b}")
                for b in range(NB)]
        ctx_sb = ctxpool.tile([P, DCH, T], FP8, tag="ctx8")

        THS = [(0, TH0)] + ([(TH0, T)] if T > TH0 else [])

        def emit_qk(wi):
            """q or k chunk wi (0-7 q, 8-15 k) -> qkT_sb[wi] bf16 [128, T]."""
            wt = wpool.tile([P, DCH, P], FP8, tag="wt", name=f"wt{wi}")
            nc.sync.dma_start(out=wt.rearrange("p c w -> p (c w)"),
                              in_=wqk8[wi])
            ps = ps2.tile([P, 1024], F32, tag="ps", name=f"qk{wi}")
            lo, hi = (0, 2) if qk_lo else (0, 1)
            for t0, t1 in THS:
                first = True
                for c in range(0, DCH, 2):
                    for li in range(lo, hi):
                        nc.tensor.matmul(
                            ps[:, t0:t1],
                            wt[:, c:c + 2, :],
                            xT_sb[:, c:c + 2, li, t0:t1],
                            start=first, stop=(c == DCH - 2 and li == hi - 1),
                            perf_mode=DR)
                        first = False
            qsc = (1.0 / np.sqrt(HD) if wi < DCH else 1.0) / WS
            if wi in (0, 8, 1, 9) and T > TH0:
                nc.scalar.activation(out=qkT_sb[wi][:, 0:TH0],
                                     in_=ps[:, 0:TH0], func=IDN,
                                     bias=bqk_sb[:, wi:wi + 1], scale=qsc)
                nc.scalar.activation(out=qkT_sb[wi][:, TH0:T],
                                     in_=ps[:, TH0:T], func=IDN,
                                     bias=bqk_sb[:, wi:wi + 1], scale=qsc)
            else:
                nc.scalar.activation(out=qkT_sb[wi], in_=ps[:, 0:T], func=IDN,
                                     bias=bqk_sb[:, wi:wi + 1], scale=qsc)

        def emit_v(b, oh, wvt, eng):
            """v half oh for block b -> v_sb[b][:, oh*512:] (bias folded)."""
            bs = slice(int(off[b]), int(off[b + 1]))
            oc = slice(oh * 512, (oh + 1) * 512)
            ps = ps1.tile([P, 512], F32, tag="cx", name=f"v{b}_{oh}")
            lo, hi = (0, 2) if v_lo else (0, 1)
            first = True
            for c in range(0, DCH, 2):
                for li in range(lo, hi):
                    nc.tensor.matmul(
                        ps[0:S[b], :],
                        xT_sb[:, c:c + 2, li, bs],
                        wvt[:, c:c + 2, :],
                        start=first, stop=(c == DCH - 2 and li == hi - 1),
                        perf_mode=DR)
                    first = False
            if eng == "act":
                nc.scalar.activation(out=v_sb[b][:, oc], in_=ps[0:S[b], :],
                                     func=CPY, scale=VS / WS)
            else:
                nc.vector.tensor_scalar(
                    out=v_sb[b][:, oc], in0=ps[0:S[b], :], scalar1=VS / WS,
                    scalar2=None, op0=mybir.AluOpType.mult)

        def emit_s2(hp, quad):
            """head-pair hp, blocks quad[0]:quad[1] (4 blocks)."""
            b0, b1 = quad
            qa, ka = qkT_sb[hp], qkT_sb[DCH + hp]
            # per-block score regions of width 2*S_b, packed to psum banks
            roffs, FW = pack_banks([2 * S[b] for b in range(b0, b1)])
            qoff = {(b, j): roffs[b - b0] + j * S[b]
                    for b in range(b0, b1) for j in range(2)}
            kpmax = max(S[b0:b1])
            sc = ps2.tile([P, 1024], F32, tag="ps", name=f"sc{hp}_{b0}")
            for b in range(b0, b1):
                bs = slice(int(off[b]), int(off[b + 1]))
                for j in range(2):
                    rows = slice(j * HD, (j + 1) * HD)
                    reg = slice(qoff[b, j], qoff[b, j] + S[b])
                    nc.tensor.matmul(sc[0:S[b], reg], ka[rows, bs],
                                     qa[rows, bs], start=True, stop=False)
                    nc.tensor.matmul(sc[0:S[b], reg],
                                     pm_sb[:, b, 0:S[b]], neg_sb[:, 0:S[b]],
                                     start=False, stop=True)
            exm = exmpool.tile([P, FWMAX], BF16, tag="exm")
            nc.scalar.activation(out=exm[0:kpmax, 0:FW], in_=sc[0:kpmax, 0:FW],
                                 func=EXP)
            # row sums over keys (partitions) -> all partitions hold sum
            sums = sumpool.tile([P, 2, CWMAX], BF16, tag="sums")
            coff = [0] * NB  # free offsets of S_b regions in cx psum
            o = 0
            for b in range(b0, b1):
                coff[b] = o
                o += S[b]
            CW = o
            for b in range(b0, b1):
                nc.gpsimd.partition_all_reduce(
                    sums[0:S[b], :, coff[b]:coff[b] + S[b]],
                    exm[0:S[b], qoff[b, 0]:qoff[b, 0] + 2 * S[b]].rearrange(
                        "k (j q) -> k j q", j=2),
                    channels=S[b], reduce_op=bass_isa.ReduceOp.add)
            rb = rbpool.tile([P, CWMAX], BF16, tag="rb")
            nc.vector.reciprocal(out=rb[0:HD, 0:CW], in_=sums[0:HD, 0, 0:CW])
            nc.vector.reciprocal(out=rb[HD:P, 0:CW], in_=sums[0:HD, 1, 0:CW])
            cx = ps1.tile([P, 512], F32, tag="cx", name=f"cx{hp}_{b0}")
            for b in range(b0, b1):
                for j in range(2):
                    h = 2 * hp + j
                    reg = slice(qoff[b, j], qoff[b, j] + S[b])
                    nc.tensor.matmul(
                        cx[j * HD:(j + 1) * HD, coff[b]:coff[b] + S[b]],
                        v_sb[b][:, h * HD:(h + 1) * HD], exm[0:S[b], reg],
                        start=True, stop=True,
                        tile_position=(0, j * HD))
            ts = slice(int(off[b0]), int(off[b1]))
            nc.vector.tensor_mul(out=ctx_sb[:, hp, ts], in0=cx[:, 0:CW],
                                 in1=rb[:, 0:CW])

        y_ts = [singles.tile([P, D], BF16, tag=f"y{m}", name=f"y_t{m}")
                for m in range(MT)]
        sys_ = [singles.tile([P, 4], F32, tag=f"sy{m}", name=f"sy{m}")
                for m in range(MT)]

        def emit_s3a(m):
            """out_proj + residual + bias + stat sums for token tile m."""
            pt = min(P, T - m * P)
            mc = slice(m * P, m * P + pt)
            ps = ps2.tile([P, 1024], F32, tag="ps", name=f"s3_{m}")
            for oh in range(2):
                oc = slice(oh * 512, (oh + 1) * 512)
                for c in range(0, DCH, 2):
                    nc.tensor.matmul(ps[0:pt, oc],
                                     ctx_sb[:, c:c + 2, mc],
                                     wo_sb[:, c:c + 2, oc],
                                     start=(c == 0), stop=False,
                                     perf_mode=DR)
                # residual: += x (hi+lo) for the 4 ident chunks of this half
                for c in range(oh * 4, oh * 4 + 4):
                    nc.tensor.matmul(ps[0:pt, c * P:(c + 1) * P],
                                     xT_sb[:, c, :, mc], id_sb,
                                     start=False, stop=False, perf_mode=DR)
                # bias row (DR: row0 = ones x bo2, row1 = zeros)
                nc.tensor.matmul(ps[0:pt, oc], ones_sb[:, :, 0:pt],
                                 bo_sb[:, :, oc], start=False, stop=True,
                                 perf_mode=DR)
            y_t, sy = y_ts[m], sys_[m]
            nc.scalar.activation(out=y_t[0:pt], in_=ps[0:pt], func=CPY,
                                 scale=1.0 / (WS * VS),
                                 accum_out=sy[0:pt, 0:1])
            scr = singles.tile([P, D], BF16, tag="scr")
            nc.vector.scalar_tensor_tensor(
                out=scr[0:pt], in0=y_t[0:pt], scalar=0.0, in1=y_t[0:pt],
                op0=mybir.AluOpType.add, op1=mybir.AluOpType.mult,
                accum_out=sy[0:pt, 1:2])

        def emit_s3b(m):
            """LayerNorm apply + store for token tile m."""
            pt = min(P, T - m * P)
            mc = slice(m * P, m * P + pt)
            y_t, sy = y_ts[m], sys_[m]
            # mu = sy/D ; var = sq/D - mu^2 ; rstd = 1/sqrt(var+eps)
            mv = smallp.tile([P, 4], F32, tag="mv")
            nc.scalar.activation(out=mv[0:pt, 0:2], in_=sy[0:pt, 0:2],
                                 func=CPY, scale=1.0 / D)
            nc.vector.tensor_tensor(out=mv[0:pt, 2:3], in0=mv[0:pt, 0:1],
                                    in1=mv[0:pt, 0:1], op=mybir.AluOpType.mult)
            nc.vector.tensor_tensor(out=mv[0:pt, 3:4], in0=mv[0:pt, 1:2],
                                    in1=mv[0:pt, 2:3],
                                    op=mybir.AluOpType.subtract)
            rstd = smallp.tile([P, 2], F32, tag="rstd")
            nc.scalar.activation(out=rstd[0:pt, 1:2], in_=mv[0:pt, 3:4],
                                 func=SQRT, bias=eps_sb[0:pt], scale=1.0)
            nc.vector.reciprocal(out=rstd[0:pt, 1:2], in_=rstd[0:pt, 1:2])
            for h in range(2):
                hc = slice(h * 512, (h + 1) * 512)
                yo = s3pool.tile([P, 512], F32, tag="yo")
                nc.vector.tensor_scalar(
                    out=yo[0:pt], in0=y_t[0:pt, hc], scalar1=mv[0:pt, 0:1],
                    scalar2=rstd[0:pt, 1:2], op0=mybir.AluOpType.subtract,
                    op1=mybir.AluOpType.mult)
                if apply_ln_affine:
                    nc.vector.tensor_mul(out=yo[0:pt], in0=yo[0:pt],
                                         in1=lnw_sb[0:pt, hc])
                    nc.vector.tensor_add(out=yo[0:pt], in0=yo[0:pt],
                                         in1=lnb_sb[0:pt, hc])
                nc.sync.dma_start(out=y[mc, hc], in_=yo[0:pt])

        # --- emission schedule: interleave S1 qk/v with S2 so the vector
        # engines start early; S3 after all S2 (ctx complete). ---
        for wi in (0, 8, 1, 9):
            emit_qk(wi)
        wv0 = wvpool.tile([P, DCH, 512], FP8, tag="wv", name="wv0")
        nc.sync.dma_start(out=wv0, in_=wv8[0])
        for b in range(NB):
            emit_v(b, 0, wv0, "act" if b % 2 else "dve")
        emit_s2(0, QUADS[0])
        emit_qk(2)
        emit_qk(10)
        emit_s2(1, QUADS[0])
        emit_qk(3)
        emit_qk(11)
        emit_s2(2, QUADS[0])
        emit_qk(4)
        emit_qk(12)
        emit_s2(3, QUADS[0])
        wv1 = wvpool.tile([P, DCH, 512], FP8, tag="wv", name="wv1")
        nc.sync.dma_start(out=wv1, in_=wv8[1])
        nc.sync.dma_start(out=xT_sb[:, :, 1, :], in_=xT8[:, :, 1, :])
        nc.sync.dma_start(out=wo_sb, in_=wo8)
        nc.sync.dma_start(out=id_sb, in_=ident8)
        for b in range(NB):
            emit_v(b, 1, wv1, "act" if b % 2 else "dve")
        emit_qk(5)
        emit_qk(13)
        emit_s2(4, QUADS[0])
        emit_qk(6)
        emit_qk(14)
        emit_s2(5, QUADS[0])
        emit_qk(7)
        emit_qk(15)
        emit_s2(6, QUADS[0])
        emit_s2(7, QUADS[0])
        emit_s2(0, QUADS[1])
        emit_s3a(0)
        emit_s2(1, QUADS[1])
        emit_s3a(1)
        for hp in range(2, 8):
            emit_s2(hp, QUADS[1])
        emit_s3b(0)
        emit_s3b(1)
        for m in range(2, MT):
            emit_s3a(m)
        for m in range(2, MT):
            emit_s3b(m)

    nc.compile()
    return nc


def _numpy_fallback(x, cluster_id, in_proj_w, in_proj_b, out_proj_w,
                    out_proj_b, ln_w, ln_b, num_heads):
    n, d = x.shape
    hd = d // num_heads
    x64 = x.astype(np.float64)
    qkv = x64 @ in_proj_w.T.astype(np.float64) + in_proj_b
    q, k, v = np.split(qkv, 3, axis=-1)
    q = q.reshape(n, num_heads, hd)
    k = k.reshape(n, num_heads, hd)
    v = v.reshape(n, num_heads, hd)
    valid = cluster_id >= 0
    allow = (cluster_id[:, None] == cluster_id[None, :]) & valid[:, None] & valid[None, :]
    scores = np.einsum("nhd,mhd->hnm", q, k) / np.sqrt(hd)
    scores = np.where(allow[None], scores, np.finfo(np.float32).min)
    scores -= scores.max(axis=-1, keepdims=True)
    e = np.exp(scores)
    attn = e / e.sum(axis=-1, keepdims=True)
    ctx = np.einsum("hnm,mhd->nhd", attn, v).reshape(n, d)
    yv = x64 + ctx @ out_proj_w.T.astype(np.float64) + out_proj_b
    mu = yv.mean(-1, keepdims=True)
    var = ((yv - mu) ** 2).mean(-1, keepdims=True)
    out = (yv - mu) / np.sqrt(var + 1e-5) * ln_w + ln_b
    return out.astype(np.float32)


def _hi_lo(a):
    hi = a.astype(NFP8)
    lo = (a - hi.astype(np.float32)).astype(NFP8)
    return hi, lo


def kernel(x, cluster_id, in_proj_w, in_proj_b, out_proj_w, out_proj_b,
           ln_w, ln_b, num_heads):
    x = np.asarray(x, dtype=np.float32)
    cid = np.asarray(cluster_id).astype(np.int64)
    in_proj_w = np.asarray(in_proj_w, dtype=np.float32)
    in_proj_b = np.asarray(in_proj_b, dtype=np.float32)
    out_proj_w = np.asarray(out_proj_w, dtype=np.float32)
    out_proj_b = np.asarray(out_proj_b, dtype=np.float32)
    ln_w = np.asarray(ln_w, dtype=np.float32)
    ln_b = np.asarray(ln_b, dtype=np.float32)
    nh = int(np.asarray(num_heads))

    counts = np.bincount(cid, minlength=NCLUST) if cid.size and cid.min() >= 0 else None
    if (x.shape != (N, D) or nh != H or counts is None
            or cid.max() >= NCLUST or counts.max() > 120):
        return _numpy_fallback(x, cid, in_proj_w, in_proj_b, out_proj_w,
                               out_proj_b, ln_w, ln_b, nh)

    # --- cluster -> (core, block) assignment: sort clusters by size desc,
    # rank 8j+c -> core c block j; S_j = max(ceil4(max size in group), 64).
    order_cl = np.argsort(-counts, kind="stable")
    assign = np.empty((NCORES, NB), dtype=np.int64)
    S = []
    for j in range(NB):
        grp = order_cl[j * NCORES:(j + 1) * NCORES]
        assign[:, j] = grp
        S.append(max(int(-(-int(counts[grp].max()) // 4) * 4), 64))
    S = tuple(S)
    off = np.concatenate([[0], np.cumsum(S)]).astype(int)
    T = int(off[-1])

    ln_trivial = bool(np.all(ln_w == 1.0) and np.all(ln_b == 0.0))
    key = (S, not ln_trivial)
    if key not in _cache:
        _cache[key] = build_program(S, not ln_trivial)
    nc = _cache[key]

    # --- shared (replicated) weight prep ---
    scale = 1.0 / np.sqrt(HD)
    wqk_t = np.ascontiguousarray(in_proj_w[:2 * D].T) * WS  # [D, 2D]
    bqk_f = in_proj_b[:2 * D].copy()
    bqk_f[:D] *= scale
    wv_t = np.ascontiguousarray(in_proj_w[2 * D:].T) * WS  # [D, D]
    bv = in_proj_b[2 * D:]
    wo_t = np.ascontiguousarray(out_proj_w.T)  # [D, D]
    bo2 = ((out_proj_b + bv @ wo_t) * (WS * VS)).astype(np.float32)

    ident = np.zeros((P, 2, P), dtype=NFP8)
    ii = np.arange(P)
    ident[ii, 0, ii] = WS * VS
    ident[ii, 1, ii] = WS * VS

    common = dict(
        wqk8=np.ascontiguousarray(
            wqk_t.reshape(DCH, P, 2 * DCH, P).transpose(2, 1, 0, 3)).astype(NFP8),
        wv8=np.ascontiguousarray(
            wv_t.reshape(DCH, P, 2, 512).transpose(2, 1, 0, 3)).astype(NFP8),
        wo8=np.ascontiguousarray(
            (wo_t * WS).reshape(DCH, P, D).transpose(1, 0, 2)).astype(NFP8),
        ident8=ident,
        bqk=np.ascontiguousarray(
            bqk_f.reshape(2 * DCH, P).T).astype(np.float32),
        bo2=np.stack([bo2, np.zeros_like(bo2)])[None].astype(NFP8),
        lnw=ln_w, lnb=ln_b)

    # token order per cluster
    sort_tok = np.argsort(cid, kind="stable")
    cl_start = np.concatenate([[0], np.cumsum(counts)]).astype(int)

    in_maps = []
    slot_tok = []
    for core in range(NCORES):
        xp = np.zeros((T, D), dtype=np.float32)
        pm = np.ones((NB, P), dtype=np.float32)
        slots = []
        toks = []
        for j in range(NB):
            cl = int(assign[core, j])
            nk = int(counts[cl])
            tk = sort_tok[cl_start[cl]:cl_start[cl] + nk]
            sl = np.arange(int(off[j]), int(off[j]) + nk)
            xp[sl] = x[tk]
            pm[j, :nk] = 0.0
            slots.append(sl)
            toks.append(tk)
        im = dict(common)
        xT = np.ascontiguousarray(xp.T)  # [D, T]
        hi, lo = _hi_lo(xT)
        x8 = np.empty((P, DCH, 2, T), dtype=NFP8)
        x8[:, :, 0, :] = hi.reshape(DCH, P, T).transpose(1, 0, 2)
        x8[:, :, 1, :] = lo.reshape(DCH, P, T).transpose(1, 0, 2)
        im["xT8"] = x8
        im["pmask"] = pm[None].astype(NBF16)
        in_maps.append(im)
        slot_tok.append((np.concatenate(slots), np.concatenate(toks)))

    res = run_bass_kernel_spmd(nc, in_maps, core_ids=list(range(NCORES)),
                               trace=TRACE)
    kernel.last_results = res

    out = np.empty((N, D), dtype=np.float32)
    for core in range(NCORES):
        slots, toks = slot_tok[core]
        out[toks] = res.results[core]["y"][slots]
    return out
